# revision 6
# baseline (speedup 1.0000x reference)
"""DeepSet election model on 8 Trainium2 NeuronCores.

Strategy (differs from the all-reduce hint, exploiting the *sorted* index):
rows are sharded by SEGMENT OWNERSHIP - core k gets every row whose election
id falls in [512k, 512(k+1)).  Every segment then lives entirely on one core,
so no collective is needed at all.

Per core pipeline (all activations bf16, accumulation f32 in PSUM):
  1. L1:   h1T[128emb, rows] = lW1.T @ xT          (xT pre-transposed on host)
  2. relu1 evac PSUM->SBUF (+lb1 bias, per-partition)        [DVE/ACT split]
  3. L2:   h2pre[rows, emb] = h1T_chunk.T @ lW2    (h1T chunk as stationary)
  4. relu2 evac PSUM->SBUF                                    [DVE/ACT split]
  5. bucket sums: constant block-ones matmul, col-tiled; rows are padded on
     the host so every 4-row bucket belongs to exactly one segment
  6. level-2: segment sums = one-hot(S2) matmul over 128-bucket chunks into
     per-window [64seg, 128] PSUM accumulators (S2 shipped from host)
  7. deferred local layer 3 (linear, pushed past the segment sum),
     global MLP, log_softmax - all on the tiny [512, 128] per-core tensor.
"""

import math
from contextlib import ExitStack

import numpy as np
import ml_dtypes

import concourse.bass as bass
import concourse.bacc as bacc
import concourse.mybir as mybir
import concourse.tile as tile
from concourse import bass_utils

BF16 = mybir.dt.bfloat16
F32 = mybir.dt.float32
AF = mybir.ActivationFunctionType
ALU = mybir.AluOpType

N_VOTERS = 1048576
NUM_ELECTIONS = 4096
C = 32     # candidates
E = 128    # embedding width
N_CORES = 8
SEGS_PER_CORE = NUM_ELECTIONS // N_CORES   # 512
W_SEGS = 64                                # segments per PSUM window
N_WINDOWS = SEGS_PER_CORE // W_SEGS        # 8
BUCKET = 4                                 # rows per bucket

_nb16 = lambda a: np.ascontiguousarray(a).astype(ml_dtypes.bfloat16)


def _build_program(w_chunks: int):
    """Build + compile the SPMD Bass program. Structure depends only on
    w_chunks (level-2 chunks per window), identical across cores."""
    n_slots = N_WINDOWS * w_chunks          # level-2 chunks (128 buckets each)
    G = n_slots                             # groups of 512 rows (1 slot/group)
    R = G * 512                             # padded rows per core
    assert G % 4 == 0, "w_chunks*8 must be divisible by 4"
    n_quads = G // 4

    nc = bacc.Bacc(
        "TRN2",
        target_bir_lowering=False,
        debug=False,
        enable_asserts=True,
        num_devices=N_CORES,
    )

    dt_in = lambda n, sh, dt: nc.dram_tensor(n, sh, dt, kind="ExternalInput").ap()
    xt4 = dt_in("xt4", [n_quads, 128, 512], BF16)
    s2d = dt_in("s2", [128, n_slots * W_SEGS], BF16)
    lw1x4 = dt_in("lw1x4", [128, E], BF16)
    lw2 = dt_in("lw2", [E, E], BF16)
    lw3 = dt_in("lw3", [E, E], BF16)
    gw1 = dt_in("gw1", [E, E], BF16)
    gw2 = dt_in("gw2", [E, E], BF16)
    gw3 = dt_in("gw3", [E, C], BF16)
    ones32 = dt_in("ones32", [128, 32], BF16)
    identb = dt_in("identb", [128, 128], BF16)
    identf = dt_in("identf", [128, 128], F32)
    lb1d = dt_in("lb1", [E, 1], F32)
    gb1d = dt_in("gb1", [E, 1], F32)
    gb2d = dt_in("gb2", [E, 1], F32)
    gb3d = dt_in("gb3", [C, 1], F32)
    out_ap = nc.dram_tensor("out", [SEGS_PER_CORE, C], F32, kind="ExternalOutput").ap()

    with tile.TileContext(nc) as tc:
        with ExitStack() as octx:
            cpool = octx.enter_context(tc.tile_pool(name="const", bufs=1))
            aggps = octx.enter_context(tc.tile_pool(name="aggps", bufs=2, space="PSUM"))
            tailp = octx.enter_context(tc.tile_pool(name="tail", bufs=1))

            # ---- resident constants ----
            def cload(ap, shape, dtype, tag):
                t = cpool.tile(shape, dtype, tag=tag)
                nc.sync.dma_start(t[:], ap[:])
                return t

            s2 = cload(s2d, [128, n_slots * W_SEGS], BF16, "s2")
            w1 = cload(lw1x4, [128, E], BF16, "w1")
            w2 = cload(lw2, [E, E], BF16, "w2")
            w3 = cload(lw3, [E, E], BF16, "w3")
            g1w = cload(gw1, [E, E], BF16, "g1w")
            g2w = cload(gw2, [E, E], BF16, "g2w")
            g3w = cload(gw3, [E, C], BF16, "g3w")
            on32 = cload(ones32, [128, 32], BF16, "on32")
            idb = cload(identb, [128, 128], BF16, "idb")
            idf = cload(identf, [128, 128], F32, "idf")
            lb1 = cload(lb1d, [E, 1], F32, "lb1")
            gb1 = cload(gb1d, [E, 1], F32, "gb1")
            gb2 = cload(gb2d, [E, 1], F32, "gb2")
            gb3 = cload(gb3d, [C, 1], F32, "gb3")

            # agg2: [128 seg, 4*128 emb] bf16 (4 chunks of 128 segments)
            agg2 = cpool.tile([128, 4 * E], BF16, tag="agg2")

            lane_flip = 0  # alternate DVE/ACT for big evacuations

            def evac_relu(dst, src, bias=None):
                nonlocal lane_flip
                lane_flip += 1
                if lane_flip % 2:
                    nc.scalar.activation(dst, src, AF.Relu,
                                         bias=bias[:] if bias is not None else 0.0)
                else:
                    if bias is not None:
                        nc.vector.tensor_scalar(dst, src, bias[:], 0.0,
                                                ALU.add, ALU.max)
                    else:
                        nc.vector.tensor_scalar_max(dst, src, 0.0)

            def evac_copy(dst, src):
                nonlocal lane_flip
                lane_flip += 1
                if lane_flip % 2:
                    nc.scalar.copy(dst, src)
                else:
                    nc.vector.tensor_copy(dst, src)

            # ================= main per-row loop =================
            with ExitStack() as ictx:
                xtp = ictx.enter_context(tc.tile_pool(name="xt", bufs=3))
                l1ps = ictx.enter_context(tc.tile_pool(name="l1ps", bufs=2, space="PSUM"))
                h1p = ictx.enter_context(tc.tile_pool(name="h1", bufs=8))
                l2ps = ictx.enter_context(tc.tile_pool(name="l2ps", bufs=2, space="PSUM"))
                h2p = ictx.enter_context(tc.tile_pool(name="h2", bufs=4))
                bkps = ictx.enter_context(tc.tile_pool(name="bkps", bufs=2, space="PSUM"))
                bksb = ictx.enter_context(tc.tile_pool(name="bksb", bufs=4))

                agg_tile = None   # current window PSUM accumulator

                for q in range(n_quads):
                    xt = xtp.tile([128, 512], BF16, tag="xt")
                    nc.sync.dma_start(xt[:], xt4[q])

                    h1s = []
                    for i in range(4):
                        l1 = l1ps.tile([128, 512], F32, tag="l1")
                        nc.tensor.matmul(
                            l1[:], w1[32 * i:32 * i + 32, :],
                            xt[32 * i:32 * i + 32, :],
                            start=True, stop=True, tile_position=(32 * i, 0),
                        )
                        h1 = h1p.tile([128, 512], BF16, tag="h1")
                        evac_relu(h1[:], l1[:], bias=lb1)
                        h1s.append(h1)

                    bk = bkps.tile([128, 512], F32, tag="bk")
                    for i in range(4):
                        h1 = h1s[i]
                        l2 = l2ps.tile([128, 512], F32, tag="l2")
                        for c4 in range(4):
                            nc.tensor.matmul(
                                l2[:, 128 * c4:128 * c4 + 128],
                                h1[:, 128 * c4:128 * c4 + 128], w2[:],
                                start=True, stop=True,
                            )
                        h2 = h2p.tile([128, 512], BF16, tag="h2")
                        evac_relu(h2[:], l2[:])
                        # bucket sums for the 4 chunks of this group
                        for c4 in range(4):
                            cid = (q * 4 + i) * 4 + c4        # global chunk id
                            j = cid % 4                       # column group
                            s_in_bank = (cid % 16) // 4       # slot within bk
                            nc.tensor.matmul(
                                bk[32 * j:32 * j + 32,
                                   128 * s_in_bank:128 * s_in_bank + 128],
                                on32[:], h2[:, 128 * c4:128 * c4 + 128],
                                start=True, stop=True, tile_position=(0, 32 * j),
                            )

                    bks = bksb.tile([128, 512], BF16, tag="bks")
                    evac_copy(bks[:], bk[:])

                    # level-2: 4 slots of 128 buckets each
                    for s4 in range(4):
                        t = q * 4 + s4
                        w = t // w_chunks
                        first = (t % w_chunks == 0)
                        last = (t % w_chunks == w_chunks - 1)
                        if first:
                            agg_tile = aggps.tile([W_SEGS, E], F32, tag="agg")
                        nc.tensor.matmul(
                            agg_tile[:],
                            s2[:, W_SEGS * t:W_SEGS * (t + 1)],
                            bks[:, 128 * s4:128 * s4 + 128],
                            start=first, stop=last,
                        )
                        if last:
                            dst = agg2[64 * (w % 2):64 * (w % 2) + 64,
                                       128 * (w // 2):128 * (w // 2) + 128]
                            nc.vector.tensor_copy(dst, agg_tile[:])

            # ---------- tail: deferred layer-3 + global MLP ----------
            with ExitStack() as tctx:
                tailps = tctx.enter_context(
                    tc.tile_pool(name="tailps", bufs=2, space="PSUM"))

                aggT = tailp.tile([128, 4 * E], BF16, tag="aggT")
                for t in range(4):
                    tp = tailps.tile([128, 128], BF16, tag="tp")
                    nc.tensor.transpose(tp[:], agg2[:, 128 * t:128 * t + 128],
                                        idb[:])
                    evac_copy(aggT[:, 128 * t:128 * t + 128], tp[:])

                def layerT(rhs_tile, w_tile, func, bias, out_dt, out_cols=E,
                           tag=""):
                    ps = tailps.tile([out_cols, 512], F32, tag="lps")
                    nc.tensor.matmul(ps[:], w_tile[:], rhs_tile[:],
                                     start=True, stop=True)
                    o = tailp.tile([out_cols, 512], out_dt, tag=tag)
                    if func is None:
                        evac_copy(o[:], ps[:])
                    else:
                        nc.scalar.activation(
                            o[:], ps[:], func,
                            bias=bias[:] if bias is not None else 0.0)
                    return o

                a3T = layerT(aggT, w3, None, None, BF16, tag="a3T")
                g1T = layerT(a3T, g1w, AF.Relu, gb1, BF16, tag="g1T")
                g2T = layerT(g1T, g2w, AF.Relu, gb2, BF16, tag="g2T")
                scT = layerT(g2T, g3w, AF.Identity, gb3, F32, out_cols=C,
                             tag="scT")

                # transpose scores back to [seg, cand] and log-softmax
                outsb = tailp.tile([128, 4 * C], F32, tag="outsb")
                for t in range(4):
                    sp = tailps.tile([128, C], F32, tag="sp")
                    nc.tensor.transpose(sp[:], scT[:, 128 * t:128 * t + 128],
                                        idf[:C, :C])
                    x = tailp.tile([128, C], F32, tag="x")
                    nc.vector.tensor_copy(x[:], sp[:])
                    mx = tailp.tile([128, 1], F32, tag="mx")
                    nc.vector.tensor_reduce(mx[:], x[:], mybir.AxisListType.X,
                                            ALU.max)
                    negmax = tailp.tile([128, 1], F32, tag="negmax")
                    nc.vector.tensor_scalar_mul(negmax[:], mx[:], -1.0)
                    ex = tailp.tile([128, C], F32, tag="ex")
                    nc.scalar.activation(ex[:], x[:], AF.Exp, bias=negmax[:])
                    ssum = tailp.tile([128, 1], F32, tag="ssum")
                    nc.vector.reduce_sum(ssum[:], ex[:],
                                         axis=mybir.AxisListType.X)
                    lse = tailp.tile([128, 1], F32, tag="lse")
                    nc.scalar.activation(lse[:], ssum[:], AF.Ln)
                    shift = tailp.tile([128, 1], F32, tag="shift")
                    nc.vector.tensor_tensor(shift[:], negmax[:], lse[:],
                                            op=ALU.subtract)
                    nc.vector.tensor_scalar_add(outsb[:, C * t:C * (t + 1)],
                                                x[:], shift[:])

                outv = out_ap.rearrange("(t p) c -> t p c", p=128)
                for t in range(4):
                    nc.sync.dma_start(outv[t], outsb[:, C * t:C * (t + 1)])

    nc.compile()
    return nc, G, R


def _prep_core(x, index_local, counts, core, w_chunks, n_quads, R):
    """Build per-core shipped tensors. index_local/x already sliced to core."""
    segs0 = core * SEGS_PER_CORE
    cnt = counts[segs0:segs0 + SEGS_PER_CORE]           # rows per owned segment
    nbuck = (cnt + BUCKET - 1) // BUCKET                # buckets per segment

    # bucket start per segment (global bucket idx, windows padded to
    # w_chunks*128 buckets each); bucket_seg = local segment of each bucket
    n_slots = N_WINDOWS * w_chunks
    bstart = np.zeros(SEGS_PER_CORE, dtype=np.int64)
    bucket_seg = np.full(n_slots * 128, -1, dtype=np.int64)
    for w in range(N_WINDOWS):
        base = w * w_chunks * 128
        sl = slice(w * W_SEGS, (w + 1) * W_SEGS)
        starts = base + np.concatenate(([0], np.cumsum(nbuck[sl])[:-1]))
        assert starts[-1] + nbuck[sl][-1] <= base + w_chunks * 128
        bstart[sl] = starts
        for s, st, nb in zip(range(sl.start, sl.stop), starts, nbuck[sl]):
            bucket_seg[st:st + nb] = s

    # scatter rows into padded layout
    seg_of_row = index_local - segs0
    ptr_local = np.concatenate(([0], np.cumsum(cnt)))
    rank = np.arange(len(index_local)) - ptr_local[seg_of_row]
    dest = bstart[seg_of_row] * BUCKET + rank
    xpad = np.zeros((R, C), dtype=np.float32)
    xpad[dest] = x
    # xt4: [n_quads, 128, 512], partition 32*i + cand
    xt4 = xpad.reshape(n_quads, 4, 512, C).transpose(0, 1, 3, 2).reshape(
        n_quads, 128, 512)

    # S2 one-hot [128, n_slots*64]: bucket (t,p) -> local seg offset in window
    slot_of_bucket = np.arange(n_slots * 128) // 128
    d = bucket_seg - (slot_of_bucket // w_chunks) * W_SEGS
    d[bucket_seg < 0] = -(10 ** 6)
    s2 = (d[:, None] == np.arange(W_SEGS)[None, :])     # [n_slots*128, 64]
    s2 = s2.reshape(n_slots, 128, W_SEGS).transpose(1, 0, 2).reshape(
        128, n_slots * W_SEGS)
    return _nb16(xt4), _nb16(s2.astype(np.float32))


def kernel(**inputs) -> np.ndarray:
    x = np.asarray(inputs["x"], dtype=np.float32)
    index = np.asarray(inputs["index"]).astype(np.int64)
    ws = {k: np.asarray(inputs[k], dtype=np.float32)
          for k in ("lW1", "lb1", "lW2", "lb2", "lW3", "lb3",
                    "gW1", "gb1", "gW2", "gb2", "gW3", "gb3")}

    # biases that we cannot wire for free must be zero (true for this model):
    # lb2 enters per-row on the free axis, lb3 would need per-segment counts.
    assert not ws["lb2"].any() and not ws["lb3"].any(), \
        "nonzero lb2/lb3 not supported by this kernel"

    if not np.all(index[:-1] <= index[1:]):
        order = np.argsort(index, kind="stable")
        index = index[order]
        x = x[order]

    counts = np.bincount(index, minlength=NUM_ELECTIONS)
    ptr = np.concatenate(([0], np.cumsum(counts)))

    # level-2 chunks per window = max buckets in any (core, window), /128, ceil
    nbuck_all = (counts + BUCKET - 1) // BUCKET
    per_win = nbuck_all.reshape(N_CORES * N_WINDOWS, W_SEGS).sum(axis=1)
    w_chunks = int(math.ceil(per_win.max() / 128.0))  # G=8*w_chunks, always %4==0

    nc, G, R = _build_program(w_chunks)
    n_quads = G // 4

    lw1x4 = np.tile(ws["lW1"], (4, 1))                   # [128, 128]
    ones32 = (np.arange(128)[:, None] // BUCKET ==
              np.arange(32)[None, :]).astype(np.float32)

    common = {
        "lw1x4": _nb16(lw1x4),
        "lw2": _nb16(ws["lW2"]),
        "lw3": _nb16(ws["lW3"]),
        "gw1": _nb16(ws["gW1"]),
        "gw2": _nb16(ws["gW2"]),
        "gw3": _nb16(ws["gW3"]),
        "ones32": _nb16(ones32),
        "identb": _nb16(np.eye(128, dtype=np.float32)),
        "identf": np.eye(128, dtype=np.float32),
        "lb1": ws["lb1"].reshape(E, 1).astype(np.float32),
        "gb1": ws["gb1"].reshape(E, 1).astype(np.float32),
        "gb2": ws["gb2"].reshape(E, 1).astype(np.float32),
        "gb3": ws["gb3"].reshape(C, 1).astype(np.float32),
    }

    in_maps = []
    for k in range(N_CORES):
        lo, hi = ptr[k * SEGS_PER_CORE], ptr[(k + 1) * SEGS_PER_CORE]
        xt4, s2 = _prep_core(x[lo:hi], index[lo:hi], counts, k,
                             w_chunks, n_quads, R)
        in_maps.append({"xt4": xt4, "s2": s2, **common})

    res = bass_utils.run_bass_kernel_spmd(nc, in_maps, core_ids=list(range(N_CORES)))
    global LAST_RESULTS, LAST_NC, LAST_IN_MAPS
    LAST_RESULTS, LAST_NC, LAST_IN_MAPS = res, nc, in_maps
    out = np.concatenate([res.results[k]["out"] for k in range(N_CORES)], axis=0)
    return out.astype(np.float32)


LAST_RESULTS = None
LAST_NC = None
LAST_IN_MAPS = None


if __name__ == "__main__":
    rng = np.random.default_rng(0)
    idx = np.sort(rng.integers(0, NUM_ELECTIONS, size=N_VOTERS)).astype(np.int64)
    demo = {
        "x": rng.standard_normal((N_VOTERS, C), dtype=np.float32),
        "index": idx,
    }
    for n, sh in (("lW1", (C, E)), ("lW2", (E, E)), ("lW3", (E, E)),
                  ("gW1", (E, E)), ("gW2", (E, E)), ("gW3", (E, C))):
        demo[n] = (rng.standard_normal(sh, dtype=np.float32) * 0.05)
    for n, sh in (("lb1", E), ("lb2", E), ("lb3", E),
                  ("gb1", E), ("gb2", E), ("gb3", C)):
        demo[n] = np.zeros(sh, np.float32)
    out = kernel(**demo)
    print(out.shape, out.dtype, np.isfinite(out).all())


# revision 7
# speedup vs baseline: 1.0008x; 1.0008x over previous
"""DeepSet election model on 8 Trainium2 NeuronCores.

Strategy (differs from the all-reduce hint, exploiting the *sorted* index):
rows are sharded by SEGMENT OWNERSHIP - core k gets every row whose election
id falls in [512k, 512(k+1)).  Every segment then lives entirely on one core,
so no collective is needed at all.

Per core pipeline (all activations bf16, accumulation f32 in PSUM):
  1. L1:   h1T[128emb, rows] = lW1.T @ xT       (xT pre-transposed on host,
           4-way row-tiled K=32 matmuls)
  2. relu1 evac PSUM->SBUF (+lb1 bias, per-partition)        [DVE/ACT split]
  3. L2:   h2pre[rows, emb] = h1T_chunk.T @ lW2 (h1T chunk as stationary)
  4. relu2 evac PSUM->SBUF                                    [DVE/ACT split]
  5. segment sums: per 128-row chunk, one-hot S_row[128rows, 64segs] matmul
     (shipped from host) accumulating into per-window [64seg, 128] PSUM;
     rows are padded per (core,window) to a fixed row count so the chunk ->
     window map is static and identical on every core
  6. deferred local layer 3 (linear, pushed past the segment sum),
     global MLP, log_softmax - all on the tiny [512, 128] per-core tensor.
"""

import math
from contextlib import ExitStack

import numpy as np
import ml_dtypes

import concourse.bass as bass
import concourse.bacc as bacc
import concourse.mybir as mybir
import concourse.tile as tile
from concourse import bass_utils

BF16 = mybir.dt.bfloat16
F32 = mybir.dt.float32
AF = mybir.ActivationFunctionType
ALU = mybir.AluOpType

N_VOTERS = 1048576
NUM_ELECTIONS = 4096
C = 32     # candidates
E = 128    # embedding width
N_CORES = 8
SEGS_PER_CORE = NUM_ELECTIONS // N_CORES   # 512
W_SEGS = 64                                # segments per PSUM window
N_WINDOWS = SEGS_PER_CORE // W_SEGS        # 8

_nb16 = lambda a: np.ascontiguousarray(a).astype(ml_dtypes.bfloat16)


def _build_program(w_rows: int):
    """Build + compile the SPMD Bass program. w_rows = padded rows per
    (core, window); multiple of 512. Identical structure on every core."""
    assert w_rows % 512 == 0
    R = N_WINDOWS * w_rows                  # rows per core
    G = R // 512                            # groups
    n_chunks = R // 128
    cpw = w_rows // 128                     # chunks per window
    assert G % 4 == 0
    n_quads = G // 4

    nc = bacc.Bacc(
        "TRN2",
        target_bir_lowering=False,
        debug=False,
        enable_asserts=True,
        num_devices=N_CORES,
    )

    dt_in = lambda n, sh, dt: nc.dram_tensor(n, sh, dt, kind="ExternalInput").ap()
    xt4 = dt_in("xt4", [n_quads, 128, 512], BF16)
    srowd = dt_in("srow", [n_quads, 128, 16 * W_SEGS], BF16)
    lw1x4 = dt_in("lw1x4", [128, E], BF16)
    lw2 = dt_in("lw2", [E, E], BF16)
    lw3 = dt_in("lw3", [E, E], BF16)
    gw1 = dt_in("gw1", [E, E], BF16)
    gw2 = dt_in("gw2", [E, E], BF16)
    gw3 = dt_in("gw3", [E, C], BF16)
    identb = dt_in("identb", [128, 128], BF16)
    identf = dt_in("identf", [128, 128], F32)
    lb1d = dt_in("lb1", [E, 1], F32)
    gb1d = dt_in("gb1", [E, 1], F32)
    gb2d = dt_in("gb2", [E, 1], F32)
    gb3d = dt_in("gb3", [C, 1], F32)
    out_ap = nc.dram_tensor("out", [SEGS_PER_CORE, C], F32, kind="ExternalOutput").ap()

    with tile.TileContext(nc) as tc:
        with ExitStack() as octx:
            cpool = octx.enter_context(tc.tile_pool(name="const", bufs=1))
            aggps = octx.enter_context(tc.tile_pool(name="aggps", bufs=1, space="PSUM"))
            tailp = octx.enter_context(tc.tile_pool(name="tail", bufs=1))

            def cload(ap, shape, dtype, tag):
                t = cpool.tile(shape, dtype, tag=tag)
                nc.sync.dma_start(t[:], ap[:])
                return t

            w1 = cload(lw1x4, [128, E], BF16, "w1")
            w2 = cload(lw2, [E, E], BF16, "w2")
            w3 = cload(lw3, [E, E], BF16, "w3")
            g1w = cload(gw1, [E, E], BF16, "g1w")
            g2w = cload(gw2, [E, E], BF16, "g2w")
            g3w = cload(gw3, [E, C], BF16, "g3w")
            idb = cload(identb, [128, 128], BF16, "idb")
            idf = cload(identf, [128, 128], F32, "idf")
            lb1 = cload(lb1d, [E, 1], F32, "lb1")
            gb1 = cload(gb1d, [E, 1], F32, "gb1")
            gb2 = cload(gb2d, [E, 1], F32, "gb2")
            gb3 = cload(gb3d, [C, 1], F32, "gb3")

            # agg2: [128 seg, 4*128 emb] bf16 (4 chunks of 128 segments)
            agg2 = cpool.tile([128, 4 * E], BF16, tag="agg2")

            lane_flip = 0

            def evac_relu(dst, src, bias=None):
                nonlocal lane_flip
                lane_flip += 1
                if lane_flip % 2:
                    nc.scalar.activation(dst, src, AF.Relu,
                                         bias=bias[:] if bias is not None else 0.0)
                else:
                    if bias is not None:
                        nc.vector.tensor_scalar(dst, src, bias[:], 0.0,
                                                ALU.add, ALU.max)
                    else:
                        nc.vector.tensor_scalar_max(dst, src, 0.0)

            def evac_copy(dst, src):
                nonlocal lane_flip
                lane_flip += 1
                if lane_flip % 2:
                    nc.scalar.copy(dst, src)
                else:
                    nc.vector.tensor_copy(dst, src)

            # ================= main per-row loop =================
            with ExitStack() as ictx:
                xtp = ictx.enter_context(tc.tile_pool(name="xt", bufs=3))
                srp = ictx.enter_context(tc.tile_pool(name="sr", bufs=3))
                l1ps = ictx.enter_context(tc.tile_pool(name="l1ps", bufs=2, space="PSUM"))
                h1p = ictx.enter_context(tc.tile_pool(name="h1", bufs=6))
                l2ps = ictx.enter_context(tc.tile_pool(name="l2ps", bufs=3, space="PSUM"))
                h2p = ictx.enter_context(tc.tile_pool(name="h2", bufs=4))

                agg_tile = None

                for q in range(n_quads):
                    xt = xtp.tile([128, 512], BF16, tag="xt")
                    nc.sync.dma_start(xt[:], xt4[q])
                    sr = srp.tile([128, 16 * W_SEGS], BF16, tag="sr")
                    nc.sync.dma_start(sr[:], srowd[q])

                    # L1: two [128,1024] PSUM tiles per quad, 2 groups each
                    h1s = []
                    for h in range(2):
                        l1 = l1ps.tile([128, 1024], F32, tag="l1")
                        for i2 in range(2):
                            i = 2 * h + i2
                            nc.tensor.matmul(
                                l1[:, 512 * i2:512 * i2 + 512],
                                w1[32 * i:32 * i + 32, :],
                                xt[32 * i:32 * i + 32, :],
                                start=True, stop=True, tile_position=(32 * i, 0),
                            )
                        h1 = h1p.tile([128, 1024], BF16, tag="h1")
                        evac_relu(h1[:], l1[:], bias=lb1)
                        h1s.append(h1)

                    # L2 + segment-sum, chunk by chunk (16 chunks per quad)
                    for i in range(4):
                        h1 = h1s[i // 2]
                        hoff = 512 * (i % 2)
                        l2 = l2ps.tile([128, 512], F32, tag="l2")
                        for c4 in range(4):
                            nc.tensor.matmul(
                                l2[:, 128 * c4:128 * c4 + 128],
                                h1[:, hoff + 128 * c4:hoff + 128 * c4 + 128],
                                w2[:],
                                start=True, stop=True,
                            )
                        h2 = h2p.tile([128, 512], BF16, tag="h2")
                        evac_relu(h2[:], l2[:])
                        for c4 in range(4):
                            cid = (q * 4 + i) * 4 + c4
                            w = cid // cpw
                            first = (cid % cpw == 0)
                            last = (cid % cpw == cpw - 1)
                            if first:
                                agg_tile = aggps.tile([W_SEGS, E], F32, tag="agg")
                            nc.tensor.matmul(
                                agg_tile[:],
                                sr[:, W_SEGS * (4 * i + c4):
                                   W_SEGS * (4 * i + c4 + 1)],
                                h2[:, 128 * c4:128 * c4 + 128],
                                start=first, stop=last,
                            )
                            if last:
                                dst = agg2[64 * (w % 2):64 * (w % 2) + 64,
                                           128 * (w // 2):128 * (w // 2) + 128]
                                nc.vector.tensor_copy(dst, agg_tile[:])

            # ---------- tail: deferred layer-3 + global MLP ----------
            with ExitStack() as tctx:
                tailps = tctx.enter_context(
                    tc.tile_pool(name="tailps", bufs=2, space="PSUM"))

                aggT = tailp.tile([128, 4 * E], BF16, tag="aggT")
                for t in range(4):
                    tp = tailps.tile([128, 128], BF16, tag="tp")
                    nc.tensor.transpose(tp[:], agg2[:, 128 * t:128 * t + 128],
                                        idb[:])
                    evac_copy(aggT[:, 128 * t:128 * t + 128], tp[:])

                def layerT(rhs_tile, w_tile, func, bias, out_dt, out_cols=E,
                           tag=""):
                    ps = tailps.tile([out_cols, 512], F32, tag="lps")
                    nc.tensor.matmul(ps[:], w_tile[:], rhs_tile[:],
                                     start=True, stop=True)
                    o = tailp.tile([out_cols, 512], out_dt, tag=tag)
                    if func is None:
                        evac_copy(o[:], ps[:])
                    else:
                        nc.scalar.activation(
                            o[:], ps[:], func,
                            bias=bias[:] if bias is not None else 0.0)
                    return o

                a3T = layerT(aggT, w3, None, None, BF16, tag="a3T")
                g1T = layerT(a3T, g1w, AF.Relu, gb1, BF16, tag="g1T")
                g2T = layerT(g1T, g2w, AF.Relu, gb2, BF16, tag="g2T")
                scT = layerT(g2T, g3w, AF.Identity, gb3, F32, out_cols=C,
                             tag="scT")

                outsb = tailp.tile([128, 4 * C], F32, tag="outsb")
                for t in range(4):
                    sp = tailps.tile([128, C], F32, tag="sp")
                    nc.tensor.transpose(sp[:], scT[:, 128 * t:128 * t + 128],
                                        idf[:C, :C])
                    x = tailp.tile([128, C], F32, tag="x")
                    nc.vector.tensor_copy(x[:], sp[:])
                    mx = tailp.tile([128, 1], F32, tag="mx")
                    nc.vector.tensor_reduce(mx[:], x[:], mybir.AxisListType.X,
                                            ALU.max)
                    negmax = tailp.tile([128, 1], F32, tag="negmax")
                    nc.vector.tensor_scalar_mul(negmax[:], mx[:], -1.0)
                    ex = tailp.tile([128, C], F32, tag="ex")
                    nc.scalar.activation(ex[:], x[:], AF.Exp, bias=negmax[:])
                    ssum = tailp.tile([128, 1], F32, tag="ssum")
                    nc.vector.reduce_sum(ssum[:], ex[:],
                                         axis=mybir.AxisListType.X)
                    lse = tailp.tile([128, 1], F32, tag="lse")
                    nc.scalar.activation(lse[:], ssum[:], AF.Ln)
                    shift = tailp.tile([128, 1], F32, tag="shift")
                    nc.vector.tensor_tensor(shift[:], negmax[:], lse[:],
                                            op=ALU.subtract)
                    nc.vector.tensor_scalar_add(outsb[:, C * t:C * (t + 1)],
                                                x[:], shift[:])

                outv = out_ap.rearrange("(t p) c -> t p c", p=128)
                for t in range(4):
                    nc.sync.dma_start(outv[t], outsb[:, C * t:C * (t + 1)])

    nc.compile()
    return nc, G, R


def _prep_core(x, index_local, counts, core, w_rows, n_quads, R):
    """Per-core xt4 + srow tensors."""
    segs0 = core * SEGS_PER_CORE
    cnt = counts[segs0:segs0 + SEGS_PER_CORE]
    seg_of_row = index_local - segs0

    # destination row: window-contiguous with per-window padding to w_rows
    win_of_row = seg_of_row // W_SEGS
    win_cnt = np.bincount(win_of_row, minlength=N_WINDOWS)
    win_orig_start = np.concatenate(([0], np.cumsum(win_cnt)[:-1]))
    dest = win_of_row * w_rows + (np.arange(len(index_local))
                                  - win_orig_start[win_of_row])
    xpad = np.zeros((R, C), dtype=np.float32)
    xpad[dest] = x
    xt4 = xpad.reshape(n_quads, 4, 512, C).transpose(0, 1, 3, 2).reshape(
        n_quads, 128, 512)

    # per-row one-hot vs window-relative segment id
    d = np.full(R, -(10 ** 6), dtype=np.int64)
    d[dest] = seg_of_row - win_of_row * W_SEGS
    srow = (d[:, None] == np.arange(W_SEGS)[None, :])      # [R, 64]
    n_chunks = R // 128
    srow = srow.reshape(n_chunks, 128, W_SEGS).transpose(1, 0, 2)
    # group 16 chunks (one quad) per DMA tile
    srow = srow.reshape(128, n_quads, 16 * W_SEGS).transpose(1, 0, 2)
    return _nb16(xt4), _nb16(np.ascontiguousarray(srow).astype(np.float32))


def kernel(**inputs) -> np.ndarray:
    x = np.asarray(inputs["x"], dtype=np.float32)
    index = np.asarray(inputs["index"]).astype(np.int64)
    ws = {k: np.asarray(inputs[k], dtype=np.float32)
          for k in ("lW1", "lb1", "lW2", "lb2", "lW3", "lb3",
                    "gW1", "gb1", "gW2", "gb2", "gW3", "gb3")}

    # lb2 enters per-row on the free axis, lb3 would need per-segment counts;
    # both are zero for this model.
    assert not ws["lb2"].any() and not ws["lb3"].any(), \
        "nonzero lb2/lb3 not supported by this kernel"

    if not np.all(index[:-1] <= index[1:]):
        order = np.argsort(index, kind="stable")
        index = index[order]
        x = x[order]

    counts = np.bincount(index, minlength=NUM_ELECTIONS)
    ptr = np.concatenate(([0], np.cumsum(counts)))

    # rows per (core, window), padded to the global max (512-aligned)
    win_rows = counts.reshape(N_CORES * N_WINDOWS, W_SEGS).sum(axis=1)
    w_rows = int(-(-win_rows.max() // 512) * 512)

    nc, G, R = _build_program(w_rows)
    n_quads = G // 4

    common = {
        "lw1x4": _nb16(np.tile(ws["lW1"], (4, 1))),
        "lw2": _nb16(ws["lW2"]),
        "lw3": _nb16(ws["lW3"]),
        "gw1": _nb16(ws["gW1"]),
        "gw2": _nb16(ws["gW2"]),
        "gw3": _nb16(ws["gW3"]),
        "identb": _nb16(np.eye(128, dtype=np.float32)),
        "identf": np.eye(128, dtype=np.float32),
        "lb1": ws["lb1"].reshape(E, 1).astype(np.float32),
        "gb1": ws["gb1"].reshape(E, 1).astype(np.float32),
        "gb2": ws["gb2"].reshape(E, 1).astype(np.float32),
        "gb3": ws["gb3"].reshape(C, 1).astype(np.float32),
    }

    in_maps = []
    for k in range(N_CORES):
        lo, hi = ptr[k * SEGS_PER_CORE], ptr[(k + 1) * SEGS_PER_CORE]
        xt4, srow = _prep_core(x[lo:hi], index[lo:hi], counts, k,
                               w_rows, n_quads, R)
        in_maps.append({"xt4": xt4, "srow": srow, **common})

    res = bass_utils.run_bass_kernel_spmd(nc, in_maps, core_ids=list(range(N_CORES)))
    global LAST_RESULTS, LAST_NC, LAST_IN_MAPS
    LAST_RESULTS, LAST_NC, LAST_IN_MAPS = res, nc, in_maps
    out = np.concatenate([res.results[k]["out"] for k in range(N_CORES)], axis=0)
    return out.astype(np.float32)


LAST_RESULTS = None
LAST_NC = None
LAST_IN_MAPS = None


if __name__ == "__main__":
    rng = np.random.default_rng(0)
    idx = np.sort(rng.integers(0, NUM_ELECTIONS, size=N_VOTERS)).astype(np.int64)
    demo = {
        "x": rng.standard_normal((N_VOTERS, C), dtype=np.float32),
        "index": idx,
    }
    for n, sh in (("lW1", (C, E)), ("lW2", (E, E)), ("lW3", (E, E)),
                  ("gW1", (E, E)), ("gW2", (E, E)), ("gW3", (E, C))):
        demo[n] = (rng.standard_normal(sh, dtype=np.float32) * 0.05)
    for n, sh in (("lb1", E), ("lb2", E), ("lb3", E),
                  ("gb1", E), ("gb2", E), ("gb3", C)):
        demo[n] = np.zeros(sh, np.float32)
    out = kernel(**demo)
    print(out.shape, out.dtype, np.isfinite(out).all())


# revision 8
# speedup vs baseline: 380.5710x; 380.2746x over previous
"""DeepSet election model on 8 Trainium2 NeuronCores.

Strategy (differs from the all-reduce hint, exploiting the *sorted* index):
rows are sharded by SEGMENT OWNERSHIP - core k gets every row whose election
id falls in [512k, 512(k+1)).  Every segment then lives entirely on one core,
so no collective is needed at all.

Per core pipeline (all activations bf16, accumulation f32 in PSUM):
  1. L1:   h1T[128emb, rows] = lW1.T @ xT       (xT pre-transposed on host,
           4-way row-tiled K=32 matmuls)
  2. relu1 evac PSUM->SBUF (+lb1 bias, per-partition)        [DVE/ACT split]
  3. L2:   h2pre[rows, emb] = h1T_chunk.T @ lW2 (h1T chunk as stationary)
  4. relu2 evac PSUM->SBUF                                    [DVE/ACT split]
  5. segment sums: per 128-row chunk, one-hot S_row[128rows, 64segs] matmul
     (shipped from host) accumulating into per-window [64seg, 128] PSUM;
     rows are padded per (core,window) to a fixed row count so the chunk ->
     window map is static and identical on every core
  6. deferred local layer 3 (linear, pushed past the segment sum),
     global MLP, log_softmax - all on the tiny [512, 128] per-core tensor.
"""

import math
from contextlib import ExitStack

import numpy as np
import ml_dtypes

import concourse.bass as bass
import concourse.bacc as bacc
import concourse.mybir as mybir
import concourse.tile as tile
from concourse import bass_utils

BF16 = mybir.dt.bfloat16
F32 = mybir.dt.float32
AF = mybir.ActivationFunctionType
ALU = mybir.AluOpType

N_VOTERS = 1048576
NUM_ELECTIONS = 4096
C = 32     # candidates
E = 128    # embedding width
N_CORES = 8
SEGS_PER_CORE = NUM_ELECTIONS // N_CORES   # 512
W_SEGS = 64                                # segments per PSUM window
N_WINDOWS = SEGS_PER_CORE // W_SEGS        # 8

_nb16 = lambda a: np.ascontiguousarray(a).astype(ml_dtypes.bfloat16)


def _build_program(w_rows: int):
    """Build + compile the SPMD Bass program. w_rows = padded rows per
    (core, window); multiple of 512. Identical structure on every core."""
    assert w_rows % 512 == 0
    R = N_WINDOWS * w_rows                  # rows per core
    G = R // 512                            # groups
    n_chunks = R // 128
    cpw = w_rows // 128                     # chunks per window
    assert G % 4 == 0
    n_quads = G // 4

    nc = bacc.Bacc(
        "TRN2",
        target_bir_lowering=False,
        debug=False,
        enable_asserts=True,
        num_devices=N_CORES,
    )

    dt_in = lambda n, sh, dt: nc.dram_tensor(n, sh, dt, kind="ExternalInput").ap()
    xt4 = dt_in("xt4", [n_quads, 128, 512], BF16)
    srowd = dt_in("srow", [n_quads, 128, 16 * W_SEGS], BF16)
    lw1x4 = dt_in("lw1x4", [128, E], BF16)
    lw2 = dt_in("lw2", [E, E], BF16)
    lw3 = dt_in("lw3", [E, E], BF16)
    gw1 = dt_in("gw1", [E, E], BF16)
    gw2 = dt_in("gw2", [E, E], BF16)
    gw3 = dt_in("gw3", [E, C], BF16)
    identb = dt_in("identb", [128, 128], BF16)
    identf = dt_in("identf", [128, 128], F32)
    lb1d = dt_in("lb1", [E, 1], F32)
    gb1d = dt_in("gb1", [E, 1], F32)
    gb2d = dt_in("gb2", [E, 1], F32)
    gb3d = dt_in("gb3", [C, 1], F32)
    out_ap = nc.dram_tensor("out", [SEGS_PER_CORE, C], F32, kind="ExternalOutput").ap()

    with tile.TileContext(nc) as tc:
        with ExitStack() as octx:
            cpool = octx.enter_context(tc.tile_pool(name="const", bufs=1))
            aggps = octx.enter_context(tc.tile_pool(name="aggps", bufs=1, space="PSUM"))
            tailp = octx.enter_context(tc.tile_pool(name="tail", bufs=1))

            def cload(ap, shape, dtype, tag):
                t = cpool.tile(shape, dtype, tag=tag)
                nc.sync.dma_start(t[:], ap[:])
                return t

            w1 = cload(lw1x4, [128, E], BF16, "w1")
            w2 = cload(lw2, [E, E], BF16, "w2")
            w3 = cload(lw3, [E, E], BF16, "w3")
            g1w = cload(gw1, [E, E], BF16, "g1w")
            g2w = cload(gw2, [E, E], BF16, "g2w")
            g3w = cload(gw3, [E, C], BF16, "g3w")
            idb = cload(identb, [128, 128], BF16, "idb")
            idf = cload(identf, [128, 128], F32, "idf")
            lb1 = cload(lb1d, [E, 1], F32, "lb1")
            gb1 = cload(gb1d, [E, 1], F32, "gb1")
            gb2 = cload(gb2d, [E, 1], F32, "gb2")
            gb3 = cload(gb3d, [C, 1], F32, "gb3")

            # agg2: [128 seg, 4*128 emb] bf16 (4 chunks of 128 segments)
            agg2 = cpool.tile([128, 4 * E], BF16, tag="agg2")

            lane_flip = 0

            def evac_relu(dst, src, bias=None):
                nonlocal lane_flip
                lane_flip += 1
                if lane_flip % 2:
                    nc.scalar.activation(dst, src, AF.Relu,
                                         bias=bias[:] if bias is not None else 0.0)
                else:
                    if bias is not None:
                        nc.vector.tensor_scalar(dst, src, bias[:], 0.0,
                                                ALU.add, ALU.max)
                    else:
                        nc.vector.tensor_scalar_max(dst, src, 0.0)

            def evac_copy(dst, src):
                nonlocal lane_flip
                lane_flip += 1
                if lane_flip % 2:
                    nc.scalar.copy(dst, src)
                else:
                    nc.vector.tensor_copy(dst, src)

            # ================= main per-row loop =================
            with ExitStack() as ictx:
                xtp = ictx.enter_context(tc.tile_pool(name="xt", bufs=4))
                srp = ictx.enter_context(tc.tile_pool(name="sr", bufs=4))
                l1ps = ictx.enter_context(tc.tile_pool(name="l1ps", bufs=2, space="PSUM"))
                h1p = ictx.enter_context(tc.tile_pool(name="h1", bufs=8))
                l2ps = ictx.enter_context(tc.tile_pool(name="l2ps", bufs=3, space="PSUM"))
                h2p = ictx.enter_context(tc.tile_pool(name="h2", bufs=6))

                agg_tile = None

                for q in range(n_quads):
                    xt = xtp.tile([128, 512], BF16, tag="xt")
                    nc.sync.dma_start(xt[:], xt4[q])
                    sr = srp.tile([128, 16 * W_SEGS], BF16, tag="sr")
                    nc.sync.dma_start(sr[:], srowd[q])

                    # L1: two [128,1024] PSUM tiles per quad, 2 groups each
                    h1s = []
                    for h in range(2):
                        l1 = l1ps.tile([128, 1024], F32, tag="l1")
                        for i2 in range(2):
                            i = 2 * h + i2
                            nc.tensor.matmul(
                                l1[:, 512 * i2:512 * i2 + 512],
                                w1[32 * i:32 * i + 32, :],
                                xt[32 * i:32 * i + 32, :],
                                start=True, stop=True, tile_position=(32 * i, 0),
                            )
                        h1 = h1p.tile([128, 1024], BF16, tag="h1")
                        evac_relu(h1[:], l1[:], bias=lb1)
                        h1s.append(h1)

                    # L2 + segment-sum, chunk by chunk (16 chunks per quad)
                    for i in range(4):
                        h1 = h1s[i // 2]
                        hoff = 512 * (i % 2)
                        l2 = l2ps.tile([128, 512], F32, tag="l2")
                        for c4 in range(4):
                            nc.tensor.matmul(
                                l2[:, 128 * c4:128 * c4 + 128],
                                h1[:, hoff + 128 * c4:hoff + 128 * c4 + 128],
                                w2[:],
                                start=True, stop=True,
                            )
                        h2 = h2p.tile([128, 512], BF16, tag="h2")
                        evac_relu(h2[:], l2[:])
                        for c4 in range(4):
                            cid = (q * 4 + i) * 4 + c4
                            w = cid // cpw
                            first = (cid % cpw == 0)
                            last = (cid % cpw == cpw - 1)
                            if first:
                                agg_tile = aggps.tile([W_SEGS, E], F32, tag="agg")
                            nc.tensor.matmul(
                                agg_tile[:],
                                sr[:, W_SEGS * (4 * i + c4):
                                   W_SEGS * (4 * i + c4 + 1)],
                                h2[:, 128 * c4:128 * c4 + 128],
                                start=first, stop=last,
                            )
                            if last:
                                dst = agg2[64 * (w % 2):64 * (w % 2) + 64,
                                           128 * (w // 2):128 * (w // 2) + 128]
                                nc.vector.tensor_copy(dst, agg_tile[:])

            # ---------- tail: deferred layer-3 + global MLP ----------
            with ExitStack() as tctx:
                tailps = tctx.enter_context(
                    tc.tile_pool(name="tailps", bufs=2, space="PSUM"))

                aggT = tailp.tile([128, 4 * E], BF16, tag="aggT")
                for t in range(4):
                    tp = tailps.tile([128, 128], BF16, tag="tp")
                    nc.tensor.transpose(tp[:], agg2[:, 128 * t:128 * t + 128],
                                        idb[:])
                    evac_copy(aggT[:, 128 * t:128 * t + 128], tp[:])

                def layerT(rhs_tile, w_tile, func, bias, out_dt, out_cols=E,
                           tag=""):
                    ps = tailps.tile([out_cols, 512], F32, tag="lps")
                    nc.tensor.matmul(ps[:], w_tile[:], rhs_tile[:],
                                     start=True, stop=True)
                    o = tailp.tile([out_cols, 512], out_dt, tag=tag)
                    if func is None:
                        evac_copy(o[:], ps[:])
                    else:
                        nc.scalar.activation(
                            o[:], ps[:], func,
                            bias=bias[:] if bias is not None else 0.0)
                    return o

                a3T = layerT(aggT, w3, None, None, BF16, tag="a3T")
                g1T = layerT(a3T, g1w, AF.Relu, gb1, BF16, tag="g1T")
                g2T = layerT(g1T, g2w, AF.Relu, gb2, BF16, tag="g2T")
                scT = layerT(g2T, g3w, AF.Identity, gb3, F32, out_cols=C,
                             tag="scT")

                outsb = tailp.tile([128, 4 * C], F32, tag="outsb")
                for t in range(4):
                    sp = tailps.tile([128, C], F32, tag="sp")
                    nc.tensor.transpose(sp[:], scT[:, 128 * t:128 * t + 128],
                                        idf[:C, :C])
                    x = tailp.tile([128, C], F32, tag="x")
                    nc.vector.tensor_copy(x[:], sp[:])
                    mx = tailp.tile([128, 1], F32, tag="mx")
                    nc.vector.tensor_reduce(mx[:], x[:], mybir.AxisListType.X,
                                            ALU.max)
                    negmax = tailp.tile([128, 1], F32, tag="negmax")
                    nc.vector.tensor_scalar_mul(negmax[:], mx[:], -1.0)
                    ex = tailp.tile([128, C], F32, tag="ex")
                    nc.scalar.activation(ex[:], x[:], AF.Exp, bias=negmax[:])
                    ssum = tailp.tile([128, 1], F32, tag="ssum")
                    nc.vector.reduce_sum(ssum[:], ex[:],
                                         axis=mybir.AxisListType.X)
                    lse = tailp.tile([128, 1], F32, tag="lse")
                    nc.scalar.activation(lse[:], ssum[:], AF.Ln)
                    shift = tailp.tile([128, 1], F32, tag="shift")
                    nc.vector.tensor_tensor(shift[:], negmax[:], lse[:],
                                            op=ALU.subtract)
                    nc.vector.tensor_scalar_add(outsb[:, C * t:C * (t + 1)],
                                                x[:], shift[:])

                outv = out_ap.rearrange("(t p) c -> t p c", p=128)
                for t in range(4):
                    nc.sync.dma_start(outv[t], outsb[:, C * t:C * (t + 1)])

    nc.compile()
    return nc, G, R


def _prep_core(x, index_local, counts, core, w_rows, n_quads, R):
    """Per-core xt4 + srow tensors."""
    segs0 = core * SEGS_PER_CORE
    cnt = counts[segs0:segs0 + SEGS_PER_CORE]
    seg_of_row = index_local - segs0

    # destination row: window-contiguous with per-window padding to w_rows
    win_of_row = seg_of_row // W_SEGS
    win_cnt = np.bincount(win_of_row, minlength=N_WINDOWS)
    win_orig_start = np.concatenate(([0], np.cumsum(win_cnt)[:-1]))
    dest = win_of_row * w_rows + (np.arange(len(index_local))
                                  - win_orig_start[win_of_row])
    xpad = np.zeros((R, C), dtype=np.float32)
    xpad[dest] = x
    xt4 = xpad.reshape(n_quads, 4, 512, C).transpose(0, 1, 3, 2).reshape(
        n_quads, 128, 512)

    # per-row one-hot vs window-relative segment id
    d = np.full(R, -(10 ** 6), dtype=np.int64)
    d[dest] = seg_of_row - win_of_row * W_SEGS
    srow = (d[:, None] == np.arange(W_SEGS)[None, :])      # [R, 64]
    n_chunks = R // 128
    srow = srow.reshape(n_chunks, 128, W_SEGS).transpose(1, 0, 2)
    # group 16 chunks (one quad) per DMA tile
    srow = srow.reshape(128, n_quads, 16 * W_SEGS).transpose(1, 0, 2)
    return _nb16(xt4), _nb16(np.ascontiguousarray(srow).astype(np.float32))


def kernel(**inputs) -> np.ndarray:
    x = np.asarray(inputs["x"], dtype=np.float32)
    index = np.asarray(inputs["index"]).astype(np.int64)
    ws = {k: np.asarray(inputs[k], dtype=np.float32)
          for k in ("lW1", "lb1", "lW2", "lb2", "lW3", "lb3",
                    "gW1", "gb1", "gW2", "gb2", "gW3", "gb3")}

    # lb2 enters per-row on the free axis, lb3 would need per-segment counts;
    # both are zero for this model.
    assert not ws["lb2"].any() and not ws["lb3"].any(), \
        "nonzero lb2/lb3 not supported by this kernel"

    if not np.all(index[:-1] <= index[1:]):
        order = np.argsort(index, kind="stable")
        index = index[order]
        x = x[order]

    counts = np.bincount(index, minlength=NUM_ELECTIONS)
    ptr = np.concatenate(([0], np.cumsum(counts)))

    # rows per (core, window), padded to the global max (512-aligned)
    win_rows = counts.reshape(N_CORES * N_WINDOWS, W_SEGS).sum(axis=1)
    w_rows = int(-(-win_rows.max() // 512) * 512)

    nc, G, R = _build_program(w_rows)
    n_quads = G // 4

    common = {
        "lw1x4": _nb16(np.tile(ws["lW1"], (4, 1))),
        "lw2": _nb16(ws["lW2"]),
        "lw3": _nb16(ws["lW3"]),
        "gw1": _nb16(ws["gW1"]),
        "gw2": _nb16(ws["gW2"]),
        "gw3": _nb16(ws["gW3"]),
        "identb": _nb16(np.eye(128, dtype=np.float32)),
        "identf": np.eye(128, dtype=np.float32),
        "lb1": ws["lb1"].reshape(E, 1).astype(np.float32),
        "gb1": ws["gb1"].reshape(E, 1).astype(np.float32),
        "gb2": ws["gb2"].reshape(E, 1).astype(np.float32),
        "gb3": ws["gb3"].reshape(C, 1).astype(np.float32),
    }

    in_maps = []
    for k in range(N_CORES):
        lo, hi = ptr[k * SEGS_PER_CORE], ptr[(k + 1) * SEGS_PER_CORE]
        xt4, srow = _prep_core(x[lo:hi], index[lo:hi], counts, k,
                               w_rows, n_quads, R)
        in_maps.append({"xt4": xt4, "srow": srow, **common})

    res = bass_utils.run_bass_kernel_spmd(nc, in_maps, core_ids=list(range(N_CORES)))
    global LAST_RESULTS, LAST_NC, LAST_IN_MAPS
    LAST_RESULTS, LAST_NC, LAST_IN_MAPS = res, nc, in_maps
    out = np.concatenate([res.results[k]["out"] for k in range(N_CORES)], axis=0)
    return out.astype(np.float32)


LAST_RESULTS = None
LAST_NC = None
LAST_IN_MAPS = None


if __name__ == "__main__":
    rng = np.random.default_rng(0)
    idx = np.sort(rng.integers(0, NUM_ELECTIONS, size=N_VOTERS)).astype(np.int64)
    demo = {
        "x": rng.standard_normal((N_VOTERS, C), dtype=np.float32),
        "index": idx,
    }
    for n, sh in (("lW1", (C, E)), ("lW2", (E, E)), ("lW3", (E, E)),
                  ("gW1", (E, E)), ("gW2", (E, E)), ("gW3", (E, C))):
        demo[n] = (rng.standard_normal(sh, dtype=np.float32) * 0.05)
    for n, sh in (("lb1", E), ("lb2", E), ("lb3", E),
                  ("gb1", E), ("gb2", E), ("gb3", C)):
        demo[n] = np.zeros(sh, np.float32)
    out = kernel(**demo)
    print(out.shape, out.dtype, np.isfinite(out).all())


# revision 11
# speedup vs baseline: 395.9991x; 1.0405x over previous
"""DeepSet election model on 8 Trainium2 NeuronCores.

Strategy (differs from the all-reduce hint, exploiting the *sorted* index):
rows are sharded by SEGMENT OWNERSHIP - core k gets every row whose election
id falls in [512k, 512(k+1)).  Every segment then lives entirely on one core,
so no collective is needed at all.

Per core pipeline (all activations bf16, accumulation f32 in PSUM):
  1. L1:   h1T[128emb, rows] = lW1.T @ xT       (xT pre-transposed on host,
           4-way row-tiled K=32 matmuls)
  2. relu1 evac PSUM->SBUF (+lb1 bias, per-partition)        [DVE/ACT split]
  3. L2:   h2pre[rows, emb] = h1T_chunk.T @ lW2 (h1T chunk as stationary)
  4. relu2 evac PSUM->SBUF                                    [DVE/ACT split]
  5. segment sums: per 128-row chunk, one-hot S_row[128rows, 64segs] matmul
     (shipped from host) accumulating into per-window [64seg, 128] PSUM;
     rows are padded per (core,window) to a fixed row count so the chunk ->
     window map is static and identical on every core
  6. deferred local layer 3 (linear, pushed past the segment sum),
     global MLP, log_softmax - all on the tiny [512, 128] per-core tensor.
"""

import math
from contextlib import ExitStack

import numpy as np
import ml_dtypes

import concourse.bass as bass
import concourse.bacc as bacc
import concourse.mybir as mybir
import concourse.tile as tile
from concourse import bass_utils

BF16 = mybir.dt.bfloat16
F32 = mybir.dt.float32
AF = mybir.ActivationFunctionType
ALU = mybir.AluOpType

N_VOTERS = 1048576
NUM_ELECTIONS = 4096
C = 32     # candidates
E = 128    # embedding width
N_CORES = 8
SEGS_PER_CORE = NUM_ELECTIONS // N_CORES   # 512
W_SEGS = 64                                # segments per PSUM window
N_WINDOWS = SEGS_PER_CORE // W_SEGS        # 8

_nb16 = lambda a: np.ascontiguousarray(a).astype(ml_dtypes.bfloat16)


def _build_program(w_rows: int):
    """Build + compile the SPMD Bass program. w_rows = padded rows per
    (core, window); multiple of 512. Identical structure on every core."""
    assert w_rows % 512 == 0
    R = N_WINDOWS * w_rows                  # rows per core
    G = R // 512                            # groups
    n_chunks = R // 128
    cpw = w_rows // 128                     # chunks per window
    assert G % 4 == 0
    n_quads = G // 4

    nc = bacc.Bacc(
        "TRN2",
        target_bir_lowering=False,
        debug=False,
        enable_asserts=True,
        num_devices=N_CORES,
    )

    dt_in = lambda n, sh, dt: nc.dram_tensor(n, sh, dt, kind="ExternalInput").ap()
    xt4 = dt_in("xt4", [n_quads, 128, 512], BF16)
    srowd = dt_in("srow", [n_quads, 128, 16 * W_SEGS], BF16)
    lw1x4 = dt_in("lw1x4", [128, E], BF16)
    lw2 = dt_in("lw2", [E, E], BF16)
    lw3 = dt_in("lw3", [E, E], BF16)
    gw1 = dt_in("gw1", [E, E], BF16)
    gw2 = dt_in("gw2", [E, E], BF16)
    gw3 = dt_in("gw3", [E, C], BF16)
    identb = dt_in("identb", [128, 128], BF16)
    identf = dt_in("identf", [128, 128], F32)
    lb1d = dt_in("lb1", [E, 1], F32)
    gb1d = dt_in("gb1", [E, 1], F32)
    gb2d = dt_in("gb2", [E, 1], F32)
    gb3d = dt_in("gb3", [C, 1], F32)
    out_ap = nc.dram_tensor("out", [SEGS_PER_CORE, C], F32, kind="ExternalOutput").ap()

    with tile.TileContext(nc) as tc:
        with ExitStack() as octx:
            cpool = octx.enter_context(tc.tile_pool(name="const", bufs=1))
            aggps = octx.enter_context(tc.tile_pool(name="aggps", bufs=1, space="PSUM"))
            tailp = octx.enter_context(tc.tile_pool(name="tail", bufs=2))

            def cload(ap, shape, dtype, tag):
                t = cpool.tile(shape, dtype, tag=tag)
                nc.sync.dma_start(t[:], ap[:])
                return t

            w1 = cload(lw1x4, [128, E], BF16, "w1")
            w2 = cload(lw2, [E, E], BF16, "w2")
            w3 = cload(lw3, [E, E], BF16, "w3")
            g1w = cload(gw1, [E, E], BF16, "g1w")
            g2w = cload(gw2, [E, E], BF16, "g2w")
            g3w = cload(gw3, [E, C], BF16, "g3w")
            idb = cload(identb, [128, 128], BF16, "idb")
            idf = cload(identf, [128, 128], F32, "idf")
            lb1 = cload(lb1d, [E, 1], F32, "lb1")
            gb1 = cload(gb1d, [E, 1], F32, "gb1")
            gb2 = cload(gb2d, [E, 1], F32, "gb2")
            gb3 = cload(gb3d, [C, 1], F32, "gb3")

            # agg2: [128 seg, 4*128 emb] bf16 (4 chunks of 128 segments)
            agg2 = cpool.tile([128, 4 * E], BF16, tag="agg2")

            lane_flip = 0

            def evac_relu(dst, src, bias=None):
                nonlocal lane_flip
                lane_flip += 1
                if lane_flip % 2:
                    nc.scalar.activation(dst, src, AF.Relu,
                                         bias=bias[:] if bias is not None else 0.0)
                else:
                    if bias is not None:
                        nc.vector.tensor_scalar(dst, src, bias[:], 0.0,
                                                ALU.add, ALU.max)
                    else:
                        nc.vector.tensor_scalar_max(dst, src, 0.0)

            def evac_copy(dst, src):
                nonlocal lane_flip
                lane_flip += 1
                if lane_flip % 2:
                    nc.scalar.copy(dst, src)
                else:
                    nc.vector.tensor_copy(dst, src)

            # ================= main per-row loop =================
            with ExitStack() as ictx:
                xtp = ictx.enter_context(tc.tile_pool(name="xt", bufs=4))
                srp = ictx.enter_context(tc.tile_pool(name="sr", bufs=4))
                l1ps = ictx.enter_context(tc.tile_pool(name="l1ps", bufs=2, space="PSUM"))
                h1p = ictx.enter_context(tc.tile_pool(name="h1", bufs=8))
                l2ps = ictx.enter_context(tc.tile_pool(name="l2ps", bufs=3, space="PSUM"))
                h2p = ictx.enter_context(tc.tile_pool(name="h2", bufs=6))

                agg_tile = None

                for q in range(n_quads):
                    xt = xtp.tile([128, 512], BF16, tag="xt")
                    nc.sync.dma_start(xt[:], xt4[q])
                    sr = srp.tile([128, 16 * W_SEGS], BF16, tag="sr")
                    nc.sync.dma_start(sr[:], srowd[q])

                    # L1: two [128,1024] PSUM tiles per quad, 2 groups each
                    h1s = []
                    for h in range(2):
                        l1 = l1ps.tile([128, 1024], F32, tag="l1")
                        for i2 in range(2):
                            i = 2 * h + i2
                            nc.tensor.matmul(
                                l1[:, 512 * i2:512 * i2 + 512],
                                w1[32 * i:32 * i + 32, :],
                                xt[32 * i:32 * i + 32, :],
                                start=True, stop=True, tile_position=(32 * i, 0),
                            )
                        h1 = h1p.tile([128, 1024], BF16, tag="h1")
                        evac_relu(h1[:], l1[:], bias=lb1)
                        h1s.append(h1)

                    # L2 + segment-sum, chunk by chunk (16 chunks per quad)
                    for i in range(4):
                        h1 = h1s[i // 2]
                        hoff = 512 * (i % 2)
                        l2 = l2ps.tile([128, 512], F32, tag="l2")
                        for c4 in range(4):
                            nc.tensor.matmul(
                                l2[:, 128 * c4:128 * c4 + 128],
                                h1[:, hoff + 128 * c4:hoff + 128 * c4 + 128],
                                w2[:],
                                start=True, stop=True,
                            )
                        h2 = h2p.tile([128, 512], BF16, tag="h2")
                        evac_relu(h2[:], l2[:])
                        for c4 in range(4):
                            cid = (q * 4 + i) * 4 + c4
                            w = cid // cpw
                            first = (cid % cpw == 0)
                            last = (cid % cpw == cpw - 1)
                            if first:
                                agg_tile = aggps.tile([W_SEGS, E], F32, tag="agg")
                            nc.tensor.matmul(
                                agg_tile[:],
                                sr[:, W_SEGS * (4 * i + c4):
                                   W_SEGS * (4 * i + c4 + 1)],
                                h2[:, 128 * c4:128 * c4 + 128],
                                start=first, stop=last,
                            )
                            if last:
                                dst = agg2[64 * (w % 2):64 * (w % 2) + 64,
                                           128 * (w // 2):128 * (w // 2) + 128]
                                nc.vector.tensor_copy(dst, agg_tile[:])

            # ---------- tail: deferred layer-3 + global MLP ----------
            with ExitStack() as tctx:
                tailps = tctx.enter_context(
                    tc.tile_pool(name="tailps", bufs=2, space="PSUM"))

                aggT = tailp.tile([128, 4 * E], BF16, tag="aggT")
                for t in range(4):
                    tp = tailps.tile([128, 128], BF16, tag="tp")
                    nc.tensor.transpose(tp[:], agg2[:, 128 * t:128 * t + 128],
                                        idb[:])
                    evac_copy(aggT[:, 128 * t:128 * t + 128], tp[:])

                def layerT(rhs_tile, w_tile, func, bias, out_dt, out_cols=E,
                           tag=""):
                    ps = tailps.tile([out_cols, 512], F32, tag="lps")
                    nc.tensor.matmul(ps[:], w_tile[:], rhs_tile[:],
                                     start=True, stop=True)
                    o = tailp.tile([out_cols, 512], out_dt, tag=tag)
                    if func is None:
                        evac_copy(o[:], ps[:])
                    else:
                        nc.scalar.activation(
                            o[:], ps[:], func,
                            bias=bias[:] if bias is not None else 0.0)
                    return o

                a3T = layerT(aggT, w3, None, None, BF16, tag="a3T")
                g1T = layerT(a3T, g1w, AF.Relu, gb1, BF16, tag="g1T")
                g2T = layerT(g1T, g2w, AF.Relu, gb2, BF16, tag="g2T")
                scT = layerT(g2T, g3w, AF.Identity, gb3, F32, out_cols=C,
                             tag="scT")

                # log-softmax, phase-batched so ACT loads the exp table once
                # and the ln table once (alternating Exp/Ln per chunk costs a
                # ~2.7us ACT table load EACH time)
                outsb = tailp.tile([128, 4 * C], F32, tag="outsb")
                xs = tailp.tile([128, 4 * C], F32, tag="xs")
                exs = tailp.tile([128, 4 * C], F32, tag="exs")
                negmax = tailp.tile([128, 4], F32, tag="negmax")
                ssum = tailp.tile([128, 4], F32, tag="ssum")
                lse = tailp.tile([128, 4], F32, tag="lse")
                shift = tailp.tile([128, 4], F32, tag="shift")
                mx = tailp.tile([128, 4], F32, tag="mx")
                for t in range(4):
                    sp = tailps.tile([128, C], F32, tag="sp")
                    nc.tensor.transpose(sp[:], scT[:, 128 * t:128 * t + 128],
                                        idf[:C, :C])
                    nc.vector.tensor_copy(xs[:, C * t:C * (t + 1)], sp[:])
                    nc.vector.tensor_reduce(mx[:, t:t + 1],
                                            xs[:, C * t:C * (t + 1)],
                                            mybir.AxisListType.X, ALU.max)
                for t in range(4):
                    nc.vector.tensor_scalar_mul(negmax[:, t:t + 1],
                                                mx[:, t:t + 1], -1.0)
                    nc.scalar.activation(exs[:, C * t:C * (t + 1)],
                                         xs[:, C * t:C * (t + 1)], AF.Exp,
                                         bias=negmax[:, t:t + 1])
                    nc.vector.reduce_sum(ssum[:, t:t + 1],
                                         exs[:, C * t:C * (t + 1)],
                                         axis=mybir.AxisListType.X)
                nc.scalar.activation(lse[:], ssum[:], AF.Ln)
                nc.vector.tensor_tensor(shift[:], negmax[:], lse[:],
                                        op=ALU.subtract)
                for t in range(4):
                    nc.vector.tensor_scalar_add(outsb[:, C * t:C * (t + 1)],
                                                xs[:, C * t:C * (t + 1)],
                                                shift[:, t:t + 1])

                outv = out_ap.rearrange("(t p) c -> t p c", p=128)
                for t in range(4):
                    nc.sync.dma_start(outv[t], outsb[:, C * t:C * (t + 1)])

    nc.compile()
    return nc, G, R


def _prep_core(x, index_local, counts, core, w_rows, n_quads, R):
    """Per-core xt4 + srow tensors."""
    segs0 = core * SEGS_PER_CORE
    cnt = counts[segs0:segs0 + SEGS_PER_CORE]
    seg_of_row = index_local - segs0

    # destination row: window-contiguous with per-window padding to w_rows
    win_of_row = seg_of_row // W_SEGS
    win_cnt = np.bincount(win_of_row, minlength=N_WINDOWS)
    win_orig_start = np.concatenate(([0], np.cumsum(win_cnt)[:-1]))
    dest = win_of_row * w_rows + (np.arange(len(index_local))
                                  - win_orig_start[win_of_row])
    xpad = np.zeros((R, C), dtype=np.float32)
    xpad[dest] = x
    xt4 = xpad.reshape(n_quads, 4, 512, C).transpose(0, 1, 3, 2).reshape(
        n_quads, 128, 512)

    # per-row one-hot vs window-relative segment id
    d = np.full(R, -(10 ** 6), dtype=np.int64)
    d[dest] = seg_of_row - win_of_row * W_SEGS
    srow = (d[:, None] == np.arange(W_SEGS)[None, :])      # [R, 64]
    n_chunks = R // 128
    srow = srow.reshape(n_chunks, 128, W_SEGS).transpose(1, 0, 2)
    # group 16 chunks (one quad) per DMA tile
    srow = srow.reshape(128, n_quads, 16 * W_SEGS).transpose(1, 0, 2)
    return _nb16(xt4), _nb16(np.ascontiguousarray(srow).astype(np.float32))


def kernel(**inputs) -> np.ndarray:
    x = np.asarray(inputs["x"], dtype=np.float32)
    index = np.asarray(inputs["index"]).astype(np.int64)
    ws = {k: np.asarray(inputs[k], dtype=np.float32)
          for k in ("lW1", "lb1", "lW2", "lb2", "lW3", "lb3",
                    "gW1", "gb1", "gW2", "gb2", "gW3", "gb3")}

    # lb2 enters per-row on the free axis, lb3 would need per-segment counts;
    # both are zero for this model.
    assert not ws["lb2"].any() and not ws["lb3"].any(), \
        "nonzero lb2/lb3 not supported by this kernel"

    if not np.all(index[:-1] <= index[1:]):
        order = np.argsort(index, kind="stable")
        index = index[order]
        x = x[order]

    counts = np.bincount(index, minlength=NUM_ELECTIONS)
    ptr = np.concatenate(([0], np.cumsum(counts)))

    # rows per (core, window), padded to the global max (512-aligned)
    win_rows = counts.reshape(N_CORES * N_WINDOWS, W_SEGS).sum(axis=1)
    w_rows = int(-(-win_rows.max() // 512) * 512)

    nc, G, R = _build_program(w_rows)
    n_quads = G // 4

    common = {
        "lw1x4": _nb16(np.tile(ws["lW1"], (4, 1))),
        "lw2": _nb16(ws["lW2"]),
        "lw3": _nb16(ws["lW3"]),
        "gw1": _nb16(ws["gW1"]),
        "gw2": _nb16(ws["gW2"]),
        "gw3": _nb16(ws["gW3"]),
        "identb": _nb16(np.eye(128, dtype=np.float32)),
        "identf": np.eye(128, dtype=np.float32),
        "lb1": ws["lb1"].reshape(E, 1).astype(np.float32),
        "gb1": ws["gb1"].reshape(E, 1).astype(np.float32),
        "gb2": ws["gb2"].reshape(E, 1).astype(np.float32),
        "gb3": ws["gb3"].reshape(C, 1).astype(np.float32),
    }

    in_maps = []
    for k in range(N_CORES):
        lo, hi = ptr[k * SEGS_PER_CORE], ptr[(k + 1) * SEGS_PER_CORE]
        xt4, srow = _prep_core(x[lo:hi], index[lo:hi], counts, k,
                               w_rows, n_quads, R)
        in_maps.append({"xt4": xt4, "srow": srow, **common})

    res = bass_utils.run_bass_kernel_spmd(nc, in_maps, core_ids=list(range(N_CORES)))
    global LAST_RESULTS, LAST_NC, LAST_IN_MAPS
    LAST_RESULTS, LAST_NC, LAST_IN_MAPS = res, nc, in_maps
    out = np.concatenate([res.results[k]["out"] for k in range(N_CORES)], axis=0)
    return out.astype(np.float32)


LAST_RESULTS = None
LAST_NC = None
LAST_IN_MAPS = None


if __name__ == "__main__":
    rng = np.random.default_rng(0)
    idx = np.sort(rng.integers(0, NUM_ELECTIONS, size=N_VOTERS)).astype(np.int64)
    demo = {
        "x": rng.standard_normal((N_VOTERS, C), dtype=np.float32),
        "index": idx,
    }
    for n, sh in (("lW1", (C, E)), ("lW2", (E, E)), ("lW3", (E, E)),
                  ("gW1", (E, E)), ("gW2", (E, E)), ("gW3", (E, C))):
        demo[n] = (rng.standard_normal(sh, dtype=np.float32) * 0.05)
    for n, sh in (("lb1", E), ("lb2", E), ("lb3", E),
                  ("gb1", E), ("gb2", E), ("gb3", C)):
        demo[n] = np.zeros(sh, np.float32)
    out = kernel(**demo)
    print(out.shape, out.dtype, np.isfinite(out).all())


# revision 13
# speedup vs baseline: 412.0410x; 1.0405x over previous
"""DeepSet election model on 8 Trainium2 NeuronCores.

Strategy (differs from the all-reduce hint, exploiting the *sorted* index):
rows are sharded by SEGMENT OWNERSHIP - core k gets every row whose election
id falls in [512k, 512(k+1)).  Every segment then lives entirely on one core,
so no collective is needed at all.

Per core pipeline (all activations bf16, accumulation f32 in PSUM):
  1. L1:   h1T[128emb, rows] = lW1.T @ xT       (xT pre-transposed on host,
           4-way row-tiled K=32 matmuls)
  2. relu1 evac PSUM->SBUF (+lb1 bias, per-partition)        [DVE/ACT split]
  3. L2:   h2pre[rows, emb] = h1T_chunk.T @ lW2 (h1T chunk as stationary)
  4. relu2 evac PSUM->SBUF                                    [DVE/ACT split]
  5. segment sums: per 128-row chunk, one-hot S_row[128rows, 64segs] matmul
     (shipped from host) accumulating into per-window [64seg, 128] PSUM;
     rows are padded per (core,window) to a fixed row count so the chunk ->
     window map is static and identical on every core
  6. deferred local layer 3 (linear, pushed past the segment sum),
     global MLP, log_softmax - all on the tiny [512, 128] per-core tensor.
"""

import math
from contextlib import ExitStack

import numpy as np
import ml_dtypes

import concourse.bass as bass
import concourse.bacc as bacc
import concourse.mybir as mybir
import concourse.tile as tile
from concourse import bass_utils

BF16 = mybir.dt.bfloat16
F32 = mybir.dt.float32
AF = mybir.ActivationFunctionType
ALU = mybir.AluOpType

N_VOTERS = 1048576
NUM_ELECTIONS = 4096
C = 32     # candidates
E = 128    # embedding width
N_CORES = 8
SEGS_PER_CORE = NUM_ELECTIONS // N_CORES   # 512
W_SEGS = 64                                # segments per PSUM window
N_WINDOWS = SEGS_PER_CORE // W_SEGS        # 8

_nb16 = lambda a: np.ascontiguousarray(a).astype(ml_dtypes.bfloat16)


def _build_program(w_rows: int):
    """Build + compile the SPMD Bass program. w_rows = padded rows per
    (core, window); multiple of 512. Identical structure on every core."""
    assert w_rows % 512 == 0
    R = N_WINDOWS * w_rows                  # rows per core
    G = R // 512                            # groups
    n_chunks = R // 128
    cpw = w_rows // 128                     # chunks per window
    assert G % 4 == 0
    n_quads = G // 4

    nc = bacc.Bacc(
        "TRN2",
        target_bir_lowering=False,
        debug=False,
        enable_asserts=True,
        num_devices=N_CORES,
    )

    dt_in = lambda n, sh, dt: nc.dram_tensor(n, sh, dt, kind="ExternalInput").ap()
    xt4 = dt_in("xt4", [n_quads, 128, 512], BF16)
    srowd = dt_in("srow", [n_quads, 128, 16 * W_SEGS], BF16)
    lw1x4 = dt_in("lw1x4", [128, E], BF16)
    lw2 = dt_in("lw2", [E, E], BF16)
    lw3 = dt_in("lw3", [E, E], BF16)
    gw1 = dt_in("gw1", [E, E], BF16)
    gw2 = dt_in("gw2", [E, E], BF16)
    gw3 = dt_in("gw3", [E, C], BF16)
    identb = dt_in("identb", [128, 128], BF16)
    identf = dt_in("identf", [128, 128], F32)
    lb1d = dt_in("lb1", [E, 1], F32)
    gb1d = dt_in("gb1", [E, 1], F32)
    gb2d = dt_in("gb2", [E, 1], F32)
    gb3d = dt_in("gb3", [C, 1], F32)
    out_ap = nc.dram_tensor("out", [SEGS_PER_CORE, C], F32, kind="ExternalOutput").ap()

    with tile.TileContext(nc) as tc:
        with ExitStack() as octx:
            cpool = octx.enter_context(tc.tile_pool(name="const", bufs=1))
            aggps = octx.enter_context(tc.tile_pool(name="aggps", bufs=1, space="PSUM"))
            tailp = octx.enter_context(tc.tile_pool(name="tail", bufs=2))

            def cload(ap, shape, dtype, tag):
                t = cpool.tile(shape, dtype, tag=tag)
                nc.sync.dma_start(t[:], ap[:])
                return t

            w1 = cload(lw1x4, [128, E], BF16, "w1")
            w2 = cload(lw2, [E, E], BF16, "w2")
            w3 = cload(lw3, [E, E], BF16, "w3")
            g1w = cload(gw1, [E, E], BF16, "g1w")
            g2w = cload(gw2, [E, E], BF16, "g2w")
            g3w = cload(gw3, [E, C], BF16, "g3w")
            idb = cload(identb, [128, 128], BF16, "idb")
            idf = cload(identf, [128, 128], F32, "idf")
            lb1 = cload(lb1d, [E, 1], F32, "lb1")
            gb1 = cload(gb1d, [E, 1], F32, "gb1")
            gb2 = cload(gb2d, [E, 1], F32, "gb2")
            gb3 = cload(gb3d, [C, 1], F32, "gb3")

            # agg2: [128 seg, 4*128 emb] bf16 (4 chunks of 128 segments)
            agg2 = cpool.tile([128, 4 * E], BF16, tag="agg2")

            lane_flip = 0
            relu2_flip = 0

            def evac_relu(dst, src, bias=None, use_act=None):
                nonlocal lane_flip
                if use_act is None:
                    lane_flip += 1
                    use_act = bool(lane_flip % 2)
                if use_act:
                    nc.scalar.activation(dst, src, AF.Relu,
                                         bias=bias[:] if bias is not None else 0.0)
                else:
                    if bias is not None:
                        nc.vector.tensor_scalar(dst, src, bias[:], 0.0,
                                                ALU.add, ALU.max)
                    else:
                        nc.vector.tensor_scalar_max(dst, src, 0.0)

            def evac_copy(dst, src):
                nonlocal lane_flip
                lane_flip += 1
                if lane_flip % 2:
                    nc.scalar.copy(dst, src)
                else:
                    nc.vector.tensor_copy(dst, src)

            # ================= main per-row loop =================
            with ExitStack() as ictx:
                xtp = ictx.enter_context(tc.tile_pool(name="xt", bufs=4))
                srp = ictx.enter_context(tc.tile_pool(name="sr", bufs=4))
                l1ps = ictx.enter_context(tc.tile_pool(name="l1ps", bufs=2, space="PSUM"))
                h1p = ictx.enter_context(tc.tile_pool(name="h1", bufs=8))
                l2ps = ictx.enter_context(tc.tile_pool(name="l2ps", bufs=3, space="PSUM"))
                h2p = ictx.enter_context(tc.tile_pool(name="h2", bufs=6))

                agg_tile = None

                for q in range(n_quads):
                    xt = xtp.tile([128, 512], BF16, tag="xt")
                    nc.sync.dma_start(xt[:], xt4[q])
                    sr = srp.tile([128, 16 * W_SEGS], BF16, tag="sr")
                    nc.sync.dma_start(sr[:], srowd[q])

                    # L1: two [128,1024] PSUM tiles per quad, 2 groups each
                    h1s = []
                    for h in range(2):
                        l1 = l1ps.tile([128, 1024], F32, tag="l1")
                        for i2 in range(2):
                            i = 2 * h + i2
                            nc.tensor.matmul(
                                l1[:, 512 * i2:512 * i2 + 512],
                                w1[32 * i:32 * i + 32, :],
                                xt[32 * i:32 * i + 32, :],
                                start=True, stop=True, tile_position=(32 * i, 0),
                            )
                        h1 = h1p.tile([128, 1024], BF16, tag="h1")
                        evac_relu(h1[:], l1[:], bias=lb1)
                        h1s.append(h1)

                    # L2 + segment-sum, chunk by chunk (16 chunks per quad)
                    for i in range(4):
                        h1 = h1s[i // 2]
                        hoff = 512 * (i % 2)
                        l2 = l2ps.tile([128, 512], F32, tag="l2")
                        for c4 in range(4):
                            nc.tensor.matmul(
                                l2[:, 128 * c4:128 * c4 + 128],
                                h1[:, hoff + 128 * c4:hoff + 128 * c4 + 128],
                                w2[:],
                                start=True, stop=True,
                            )
                        h2 = h2p.tile([128, 512], BF16, tag="h2")
                        # ACT is ~1.15x faster per op than DVE at 1x; a 4:3
                        # ACT:DVE split on relu2 (with relu1 at 1:1) equalizes
                        # total lane busy (~152us each vs 165/141 at 1:1)
                        relu2_flip += 1
                        evac_relu(h2[:], l2[:],
                                  use_act=(relu2_flip % 7) in (1, 3, 5, 6))
                        for c4 in range(4):
                            cid = (q * 4 + i) * 4 + c4
                            w = cid // cpw
                            first = (cid % cpw == 0)
                            last = (cid % cpw == cpw - 1)
                            if first:
                                agg_tile = aggps.tile([W_SEGS, E], F32, tag="agg")
                            nc.tensor.matmul(
                                agg_tile[:],
                                sr[:, W_SEGS * (4 * i + c4):
                                   W_SEGS * (4 * i + c4 + 1)],
                                h2[:, 128 * c4:128 * c4 + 128],
                                start=first, stop=last,
                            )
                            if last:
                                dst = agg2[64 * (w % 2):64 * (w % 2) + 64,
                                           128 * (w // 2):128 * (w // 2) + 128]
                                nc.vector.tensor_copy(dst, agg_tile[:])

            # ---------- tail: deferred layer-3 + global MLP ----------
            with ExitStack() as tctx:
                tailps = tctx.enter_context(
                    tc.tile_pool(name="tailps", bufs=2, space="PSUM"))

                aggT = tailp.tile([128, 4 * E], BF16, tag="aggT")
                for t in range(4):
                    tp = tailps.tile([128, 128], BF16, tag="tp")
                    nc.tensor.transpose(tp[:], agg2[:, 128 * t:128 * t + 128],
                                        idb[:])
                    evac_copy(aggT[:, 128 * t:128 * t + 128], tp[:])

                def layerT(rhs_tile, w_tile, func, bias, out_dt, out_cols=E,
                           tag=""):
                    ps = tailps.tile([out_cols, 512], F32, tag="lps")
                    nc.tensor.matmul(ps[:], w_tile[:], rhs_tile[:],
                                     start=True, stop=True)
                    o = tailp.tile([out_cols, 512], out_dt, tag=tag)
                    if func is None:
                        evac_copy(o[:], ps[:])
                    else:
                        nc.scalar.activation(
                            o[:], ps[:], func,
                            bias=bias[:] if bias is not None else 0.0)
                    return o

                a3T = layerT(aggT, w3, None, None, BF16, tag="a3T")
                g1T = layerT(a3T, g1w, AF.Relu, gb1, BF16, tag="g1T")
                g2T = layerT(g1T, g2w, AF.Relu, gb2, BF16, tag="g2T")
                scT = layerT(g2T, g3w, AF.Identity, gb3, F32, out_cols=C,
                             tag="scT")

                # log-softmax, phase-batched so ACT loads the exp table once
                # and the ln table once (alternating Exp/Ln per chunk costs a
                # ~2.7us ACT table load EACH time)
                outsb = tailp.tile([128, 4 * C], F32, tag="outsb")
                xs = tailp.tile([128, 4 * C], F32, tag="xs")
                exs = tailp.tile([128, 4 * C], F32, tag="exs")
                negmax = tailp.tile([128, 4], F32, tag="negmax")
                ssum = tailp.tile([128, 4], F32, tag="ssum")
                lse = tailp.tile([128, 4], F32, tag="lse")
                shift = tailp.tile([128, 4], F32, tag="shift")
                mx = tailp.tile([128, 4], F32, tag="mx")
                for t in range(4):
                    sp = tailps.tile([128, C], F32, tag="sp")
                    nc.tensor.transpose(sp[:], scT[:, 128 * t:128 * t + 128],
                                        idf[:C, :C])
                    nc.vector.tensor_copy(xs[:, C * t:C * (t + 1)], sp[:])
                    nc.vector.tensor_reduce(mx[:, t:t + 1],
                                            xs[:, C * t:C * (t + 1)],
                                            mybir.AxisListType.X, ALU.max)
                for t in range(4):
                    nc.vector.tensor_scalar_mul(negmax[:, t:t + 1],
                                                mx[:, t:t + 1], -1.0)
                    nc.scalar.activation(exs[:, C * t:C * (t + 1)],
                                         xs[:, C * t:C * (t + 1)], AF.Exp,
                                         bias=negmax[:, t:t + 1])
                    nc.vector.reduce_sum(ssum[:, t:t + 1],
                                         exs[:, C * t:C * (t + 1)],
                                         axis=mybir.AxisListType.X)
                nc.scalar.activation(lse[:], ssum[:], AF.Ln)
                nc.vector.tensor_tensor(shift[:], negmax[:], lse[:],
                                        op=ALU.subtract)
                for t in range(4):
                    nc.vector.tensor_scalar_add(outsb[:, C * t:C * (t + 1)],
                                                xs[:, C * t:C * (t + 1)],
                                                shift[:, t:t + 1])

                outv = out_ap.rearrange("(t p) c -> t p c", p=128)
                for t in range(4):
                    nc.sync.dma_start(outv[t], outsb[:, C * t:C * (t + 1)])

    nc.compile()
    return nc, G, R


def _prep_core(x, index_local, counts, core, w_rows, n_quads, R):
    """Per-core xt4 + srow tensors."""
    segs0 = core * SEGS_PER_CORE
    cnt = counts[segs0:segs0 + SEGS_PER_CORE]
    seg_of_row = index_local - segs0

    # destination row: window-contiguous with per-window padding to w_rows
    win_of_row = seg_of_row // W_SEGS
    win_cnt = np.bincount(win_of_row, minlength=N_WINDOWS)
    win_orig_start = np.concatenate(([0], np.cumsum(win_cnt)[:-1]))
    dest = win_of_row * w_rows + (np.arange(len(index_local))
                                  - win_orig_start[win_of_row])
    xpad = np.zeros((R, C), dtype=np.float32)
    xpad[dest] = x
    xt4 = xpad.reshape(n_quads, 4, 512, C).transpose(0, 1, 3, 2).reshape(
        n_quads, 128, 512)

    # per-row one-hot vs window-relative segment id
    d = np.full(R, -(10 ** 6), dtype=np.int64)
    d[dest] = seg_of_row - win_of_row * W_SEGS
    srow = (d[:, None] == np.arange(W_SEGS)[None, :])      # [R, 64]
    n_chunks = R // 128
    srow = srow.reshape(n_chunks, 128, W_SEGS).transpose(1, 0, 2)
    # group 16 chunks (one quad) per DMA tile
    srow = srow.reshape(128, n_quads, 16 * W_SEGS).transpose(1, 0, 2)
    return _nb16(xt4), _nb16(np.ascontiguousarray(srow).astype(np.float32))


def kernel(**inputs) -> np.ndarray:
    x = np.asarray(inputs["x"], dtype=np.float32)
    index = np.asarray(inputs["index"]).astype(np.int64)
    ws = {k: np.asarray(inputs[k], dtype=np.float32)
          for k in ("lW1", "lb1", "lW2", "lb2", "lW3", "lb3",
                    "gW1", "gb1", "gW2", "gb2", "gW3", "gb3")}

    # lb2 enters per-row on the free axis, lb3 would need per-segment counts;
    # both are zero for this model.
    assert not ws["lb2"].any() and not ws["lb3"].any(), \
        "nonzero lb2/lb3 not supported by this kernel"

    if not np.all(index[:-1] <= index[1:]):
        order = np.argsort(index, kind="stable")
        index = index[order]
        x = x[order]

    counts = np.bincount(index, minlength=NUM_ELECTIONS)
    ptr = np.concatenate(([0], np.cumsum(counts)))

    # rows per (core, window), padded to the global max (512-aligned)
    win_rows = counts.reshape(N_CORES * N_WINDOWS, W_SEGS).sum(axis=1)
    w_rows = int(-(-win_rows.max() // 512) * 512)

    nc, G, R = _build_program(w_rows)
    n_quads = G // 4

    common = {
        "lw1x4": _nb16(np.tile(ws["lW1"], (4, 1))),
        "lw2": _nb16(ws["lW2"]),
        "lw3": _nb16(ws["lW3"]),
        "gw1": _nb16(ws["gW1"]),
        "gw2": _nb16(ws["gW2"]),
        "gw3": _nb16(ws["gW3"]),
        "identb": _nb16(np.eye(128, dtype=np.float32)),
        "identf": np.eye(128, dtype=np.float32),
        "lb1": ws["lb1"].reshape(E, 1).astype(np.float32),
        "gb1": ws["gb1"].reshape(E, 1).astype(np.float32),
        "gb2": ws["gb2"].reshape(E, 1).astype(np.float32),
        "gb3": ws["gb3"].reshape(C, 1).astype(np.float32),
    }

    in_maps = []
    for k in range(N_CORES):
        lo, hi = ptr[k * SEGS_PER_CORE], ptr[(k + 1) * SEGS_PER_CORE]
        xt4, srow = _prep_core(x[lo:hi], index[lo:hi], counts, k,
                               w_rows, n_quads, R)
        in_maps.append({"xt4": xt4, "srow": srow, **common})

    res = bass_utils.run_bass_kernel_spmd(nc, in_maps, core_ids=list(range(N_CORES)))
    global LAST_RESULTS, LAST_NC, LAST_IN_MAPS
    LAST_RESULTS, LAST_NC, LAST_IN_MAPS = res, nc, in_maps
    out = np.concatenate([res.results[k]["out"] for k in range(N_CORES)], axis=0)
    return out.astype(np.float32)


LAST_RESULTS = None
LAST_NC = None
LAST_IN_MAPS = None


if __name__ == "__main__":
    rng = np.random.default_rng(0)
    idx = np.sort(rng.integers(0, NUM_ELECTIONS, size=N_VOTERS)).astype(np.int64)
    demo = {
        "x": rng.standard_normal((N_VOTERS, C), dtype=np.float32),
        "index": idx,
    }
    for n, sh in (("lW1", (C, E)), ("lW2", (E, E)), ("lW3", (E, E)),
                  ("gW1", (E, E)), ("gW2", (E, E)), ("gW3", (E, C))):
        demo[n] = (rng.standard_normal(sh, dtype=np.float32) * 0.05)
    for n, sh in (("lb1", E), ("lb2", E), ("lb3", E),
                  ("gb1", E), ("gb2", E), ("gb3", C)):
        demo[n] = np.zeros(sh, np.float32)
    out = kernel(**demo)
    print(out.shape, out.dtype, np.isfinite(out).all())


# revision 16
# speedup vs baseline: 420.7293x; 1.0211x over previous
"""DeepSet election model on 8 Trainium2 NeuronCores.

Strategy (differs from the all-reduce hint, exploiting the *sorted* index):
rows are sharded by SEGMENT OWNERSHIP - core k gets every row whose election
id falls in [512k, 512(k+1)).  Every segment then lives entirely on one core,
so no collective is needed at all.

Per core pipeline (all activations bf16, accumulation f32 in PSUM):
  1. L1:   h1T[128emb, rows] = lW1.T @ xT       (xT pre-transposed on host,
           4-way row-tiled K=32 matmuls)
  2. relu1 evac PSUM->SBUF (+lb1 bias, per-partition)        [DVE/ACT split]
  3. L2:   h2pre[rows, emb] = h1T_chunk.T @ lW2 (h1T chunk as stationary)
  4. relu2 evac PSUM->SBUF                                    [DVE/ACT split]
  5. segment sums: per 128-row chunk, one-hot S_row[128rows, 64segs] matmul
     (shipped from host) accumulating into per-window [64seg, 128] PSUM;
     rows are padded per (core,window) to a fixed row count so the chunk ->
     window map is static and identical on every core
  6. deferred local layer 3 (linear, pushed past the segment sum),
     global MLP, log_softmax - all on the tiny [512, 128] per-core tensor.
"""

import math
from contextlib import ExitStack

import numpy as np
import ml_dtypes

import concourse.bass as bass
import concourse.bacc as bacc
import concourse.mybir as mybir
import concourse.tile as tile
from concourse import bass_utils

BF16 = mybir.dt.bfloat16
F32 = mybir.dt.float32
AF = mybir.ActivationFunctionType
ALU = mybir.AluOpType

N_VOTERS = 1048576
NUM_ELECTIONS = 4096
C = 32     # candidates
E = 128    # embedding width
N_CORES = 8
SEGS_PER_CORE = NUM_ELECTIONS // N_CORES   # 512
W_SEGS = 64                                # segments per PSUM window
N_WINDOWS = SEGS_PER_CORE // W_SEGS        # 8

_nb16 = lambda a: np.ascontiguousarray(a).astype(ml_dtypes.bfloat16)


def _build_program(w_rows: int):
    """Build + compile the SPMD Bass program. w_rows = padded rows per
    (core, window); multiple of 512. Identical structure on every core."""
    assert w_rows % 512 == 0
    R = N_WINDOWS * w_rows                  # rows per core
    G = R // 512                            # groups
    n_chunks = R // 128
    cpw = w_rows // 128                     # chunks per window
    assert G % 4 == 0
    n_quads = G // 4

    nc = bacc.Bacc(
        "TRN2",
        target_bir_lowering=False,
        debug=False,
        enable_asserts=True,
        num_devices=N_CORES,
    )

    dt_in = lambda n, sh, dt: nc.dram_tensor(n, sh, dt, kind="ExternalInput").ap()
    xt4 = dt_in("xt4", [n_quads, 128, 512], BF16)
    srowd = dt_in("srow", [n_quads, 128, 16 * W_SEGS], BF16)
    lw1x4 = dt_in("lw1x4", [128, E], BF16)
    lw2 = dt_in("lw2", [E, E], BF16)
    lw3 = dt_in("lw3", [E, E], BF16)
    gw1 = dt_in("gw1", [E, E], BF16)
    gw2 = dt_in("gw2", [E, E], BF16)
    gw3 = dt_in("gw3", [E, C], BF16)
    identb = dt_in("identb", [128, 128], BF16)
    identf = dt_in("identf", [128, 128], F32)
    lb1d = dt_in("lb1", [E, 1], F32)
    gb1d = dt_in("gb1", [E, 1], F32)
    gb2d = dt_in("gb2", [E, 1], F32)
    gb3d = dt_in("gb3", [C, 1], F32)
    out_ap = nc.dram_tensor("out", [SEGS_PER_CORE, C], F32, kind="ExternalOutput").ap()

    with tile.TileContext(nc) as tc:
        with ExitStack() as octx:
            cpool = octx.enter_context(tc.tile_pool(name="const", bufs=1))
            aggps = octx.enter_context(tc.tile_pool(name="aggps", bufs=1, space="PSUM"))
            tailp = octx.enter_context(tc.tile_pool(name="tail", bufs=2))

            def cload(ap, shape, dtype, tag):
                t = cpool.tile(shape, dtype, tag=tag)
                nc.sync.dma_start(t[:], ap[:])
                return t

            w1 = cload(lw1x4, [128, E], BF16, "w1")
            w2 = cload(lw2, [E, E], BF16, "w2")
            w3 = cload(lw3, [E, E], BF16, "w3")
            g1w = cload(gw1, [E, E], BF16, "g1w")
            g2w = cload(gw2, [E, E], BF16, "g2w")
            g3w = cload(gw3, [E, C], BF16, "g3w")
            idb = cload(identb, [128, 128], BF16, "idb")
            idf = cload(identf, [128, 128], F32, "idf")
            lb1 = cload(lb1d, [E, 1], F32, "lb1")
            gb1 = cload(gb1d, [E, 1], F32, "gb1")
            gb2 = cload(gb2d, [E, 1], F32, "gb2")
            gb3 = cload(gb3d, [C, 1], F32, "gb3")

            # aggT: [128 emb, 512 seg] bf16 - segment sums, already transposed
            # for the tail (agg matmul keeps h2 stationary and streams S_row,
            # so PSUM gets aggT windows [128 emb, 64 seg] directly)
            aggT = cpool.tile([128, 4 * E], BF16, tag="aggT")

            lane_flip = 0
            relu2_flip = 0

            def evac_relu(dst, src, bias=None, use_act=None):
                nonlocal lane_flip
                if use_act is None:
                    lane_flip += 1
                    use_act = bool(lane_flip % 2)
                if use_act:
                    nc.scalar.activation(dst, src, AF.Relu,
                                         bias=bias[:] if bias is not None else 0.0)
                else:
                    if bias is not None:
                        nc.vector.tensor_scalar(dst, src, bias[:], 0.0,
                                                ALU.add, ALU.max)
                    else:
                        nc.vector.tensor_scalar_max(dst, src, 0.0)

            def evac_copy(dst, src):
                nonlocal lane_flip
                lane_flip += 1
                if lane_flip % 2:
                    nc.scalar.copy(dst, src)
                else:
                    nc.vector.tensor_copy(dst, src)

            # ================= main per-row loop =================
            with ExitStack() as ictx:
                xtp = ictx.enter_context(tc.tile_pool(name="xt", bufs=4))
                srp = ictx.enter_context(tc.tile_pool(name="sr", bufs=4))
                l1ps = ictx.enter_context(tc.tile_pool(name="l1ps", bufs=2, space="PSUM"))
                h1p = ictx.enter_context(tc.tile_pool(name="h1", bufs=8))
                l2ps = ictx.enter_context(tc.tile_pool(name="l2ps", bufs=3, space="PSUM"))
                h2p = ictx.enter_context(tc.tile_pool(name="h2", bufs=6))

                agg_tile = None

                for q in range(n_quads):
                    xt = xtp.tile([128, 512], BF16, tag="xt")
                    nc.sync.dma_start(xt[:], xt4[q])
                    sr = srp.tile([128, 16 * W_SEGS], BF16, tag="sr")
                    nc.sync.dma_start(sr[:], srowd[q])

                    # L1: two [128,1024] PSUM tiles per quad, 2 groups each
                    h1s = []
                    for h in range(2):
                        l1 = l1ps.tile([128, 1024], F32, tag="l1")
                        for i2 in range(2):
                            i = 2 * h + i2
                            nc.tensor.matmul(
                                l1[:, 512 * i2:512 * i2 + 512],
                                w1[32 * i:32 * i + 32, :],
                                xt[32 * i:32 * i + 32, :],
                                start=True, stop=True, tile_position=(32 * i, 0),
                            )
                        h1 = h1p.tile([128, 1024], BF16, tag="h1")
                        evac_relu(h1[:], l1[:], bias=lb1)
                        h1s.append(h1)

                    # L2 + segment-sum, chunk by chunk (16 chunks per quad)
                    for i in range(4):
                        h1 = h1s[i // 2]
                        hoff = 512 * (i % 2)
                        l2 = l2ps.tile([128, 512], F32, tag="l2")
                        for c4 in range(4):
                            nc.tensor.matmul(
                                l2[:, 128 * c4:128 * c4 + 128],
                                h1[:, hoff + 128 * c4:hoff + 128 * c4 + 128],
                                w2[:],
                                start=True, stop=True,
                            )
                        h2 = h2p.tile([128, 512], BF16, tag="h2")
                        # ACT is ~1.15x faster per op than DVE at 1x; a 4:3
                        # ACT:DVE split on relu2 (with relu1 at 1:1) equalizes
                        # total lane busy (~152us each vs 165/141 at 1:1)
                        relu2_flip += 1
                        evac_relu(h2[:], l2[:],
                                  use_act=(relu2_flip % 7) in (1, 3, 5, 6))
                        for c4 in range(4):
                            cid = (q * 4 + i) * 4 + c4
                            w = cid // cpw
                            first = (cid % cpw == 0)
                            last = (cid % cpw == cpw - 1)
                            if first:
                                agg_tile = aggps.tile([E, W_SEGS], F32, tag="agg")
                            # aggT_w = h2_chunk.T @ S_row: h2 stationary
                            # (FWL-eligible), S_row streamed at N=64 (half the
                            # stream of the S_row-stationary form), and the
                            # output lands pre-transposed for the tail
                            nc.tensor.matmul(
                                agg_tile[:],
                                h2[:, 128 * c4:128 * c4 + 128],
                                sr[:, W_SEGS * (4 * i + c4):
                                   W_SEGS * (4 * i + c4 + 1)],
                                start=first, stop=last,
                            )
                            if last:
                                nc.vector.tensor_copy(
                                    aggT[:, W_SEGS * w:W_SEGS * (w + 1)],
                                    agg_tile[:])

            # ---------- tail: deferred layer-3 + global MLP ----------
            with ExitStack() as tctx:
                tailps = tctx.enter_context(
                    tc.tile_pool(name="tailps", bufs=2, space="PSUM"))

                def layerT(rhs_tile, w_tile, func, bias, out_dt, out_cols=E,
                           tag=""):
                    ps = tailps.tile([out_cols, 512], F32, tag="lps")
                    nc.tensor.matmul(ps[:], w_tile[:], rhs_tile[:],
                                     start=True, stop=True)
                    o = tailp.tile([out_cols, 512], out_dt, tag=tag)
                    if func is None:
                        evac_copy(o[:], ps[:])
                    else:
                        nc.scalar.activation(
                            o[:], ps[:], func,
                            bias=bias[:] if bias is not None else 0.0)
                    return o

                a3T = layerT(aggT, w3, None, None, BF16, tag="a3T")
                g1T = layerT(a3T, g1w, AF.Relu, gb1, BF16, tag="g1T")
                g2T = layerT(g1T, g2w, AF.Relu, gb2, BF16, tag="g2T")
                scT = layerT(g2T, g3w, AF.Identity, gb3, F32, out_cols=C,
                             tag="scT")

                # log-softmax, phase-batched so ACT loads the exp table once
                # and the ln table once (alternating Exp/Ln per chunk costs a
                # ~2.7us ACT table load EACH time)
                outsb = tailp.tile([128, 4 * C], F32, tag="outsb")
                xs = tailp.tile([128, 4 * C], F32, tag="xs")
                exs = tailp.tile([128, 4 * C], F32, tag="exs")
                negmax = tailp.tile([128, 4], F32, tag="negmax")
                ssum = tailp.tile([128, 4], F32, tag="ssum")
                lse = tailp.tile([128, 4], F32, tag="lse")
                shift = tailp.tile([128, 4], F32, tag="shift")
                mx = tailp.tile([128, 4], F32, tag="mx")
                for t in range(4):
                    sp = tailps.tile([128, C], F32, tag="sp")
                    nc.tensor.transpose(sp[:], scT[:, 128 * t:128 * t + 128],
                                        idf[:C, :C])
                    nc.vector.tensor_copy(xs[:, C * t:C * (t + 1)], sp[:])
                    nc.vector.tensor_reduce(mx[:, t:t + 1],
                                            xs[:, C * t:C * (t + 1)],
                                            mybir.AxisListType.X, ALU.max)
                for t in range(4):
                    nc.vector.tensor_scalar_mul(negmax[:, t:t + 1],
                                                mx[:, t:t + 1], -1.0)
                    nc.scalar.activation(exs[:, C * t:C * (t + 1)],
                                         xs[:, C * t:C * (t + 1)], AF.Exp,
                                         bias=negmax[:, t:t + 1])
                    nc.vector.reduce_sum(ssum[:, t:t + 1],
                                         exs[:, C * t:C * (t + 1)],
                                         axis=mybir.AxisListType.X)
                nc.scalar.activation(lse[:], ssum[:], AF.Ln)
                nc.vector.tensor_tensor(shift[:], negmax[:], lse[:],
                                        op=ALU.subtract)
                for t in range(4):
                    nc.vector.tensor_scalar_add(outsb[:, C * t:C * (t + 1)],
                                                xs[:, C * t:C * (t + 1)],
                                                shift[:, t:t + 1])

                outv = out_ap.rearrange("(t p) c -> t p c", p=128)
                for t in range(4):
                    nc.sync.dma_start(outv[t], outsb[:, C * t:C * (t + 1)])

    nc.compile()
    return nc, G, R


def _prep_core(x, index_local, counts, core, w_rows, n_quads, R):
    """Per-core xt4 + srow tensors."""
    segs0 = core * SEGS_PER_CORE
    cnt = counts[segs0:segs0 + SEGS_PER_CORE]
    seg_of_row = index_local - segs0

    # destination row: window-contiguous with per-window padding to w_rows
    win_of_row = seg_of_row // W_SEGS
    win_cnt = np.bincount(win_of_row, minlength=N_WINDOWS)
    win_orig_start = np.concatenate(([0], np.cumsum(win_cnt)[:-1]))
    dest = win_of_row * w_rows + (np.arange(len(index_local))
                                  - win_orig_start[win_of_row])
    xpad = np.zeros((R, C), dtype=np.float32)
    xpad[dest] = x
    xt4 = xpad.reshape(n_quads, 4, 512, C).transpose(0, 1, 3, 2).reshape(
        n_quads, 128, 512)

    # per-row one-hot vs window-relative segment id
    d = np.full(R, -(10 ** 6), dtype=np.int64)
    d[dest] = seg_of_row - win_of_row * W_SEGS
    srow = (d[:, None] == np.arange(W_SEGS)[None, :])      # [R, 64]
    n_chunks = R // 128
    srow = srow.reshape(n_chunks, 128, W_SEGS).transpose(1, 0, 2)
    # group 16 chunks (one quad) per DMA tile
    srow = srow.reshape(128, n_quads, 16 * W_SEGS).transpose(1, 0, 2)
    return _nb16(xt4), _nb16(np.ascontiguousarray(srow).astype(np.float32))


def kernel(**inputs) -> np.ndarray:
    x = np.asarray(inputs["x"], dtype=np.float32)
    index = np.asarray(inputs["index"]).astype(np.int64)
    ws = {k: np.asarray(inputs[k], dtype=np.float32)
          for k in ("lW1", "lb1", "lW2", "lb2", "lW3", "lb3",
                    "gW1", "gb1", "gW2", "gb2", "gW3", "gb3")}

    # lb2 enters per-row on the free axis, lb3 would need per-segment counts;
    # both are zero for this model.
    assert not ws["lb2"].any() and not ws["lb3"].any(), \
        "nonzero lb2/lb3 not supported by this kernel"

    if not np.all(index[:-1] <= index[1:]):
        order = np.argsort(index, kind="stable")
        index = index[order]
        x = x[order]

    counts = np.bincount(index, minlength=NUM_ELECTIONS)
    ptr = np.concatenate(([0], np.cumsum(counts)))

    # rows per (core, window), padded to the global max (512-aligned)
    win_rows = counts.reshape(N_CORES * N_WINDOWS, W_SEGS).sum(axis=1)
    w_rows = int(-(-win_rows.max() // 512) * 512)

    nc, G, R = _build_program(w_rows)
    n_quads = G // 4

    common = {
        "lw1x4": _nb16(np.tile(ws["lW1"], (4, 1))),
        "lw2": _nb16(ws["lW2"]),
        "lw3": _nb16(ws["lW3"]),
        "gw1": _nb16(ws["gW1"]),
        "gw2": _nb16(ws["gW2"]),
        "gw3": _nb16(ws["gW3"]),
        "identb": _nb16(np.eye(128, dtype=np.float32)),
        "identf": np.eye(128, dtype=np.float32),
        "lb1": ws["lb1"].reshape(E, 1).astype(np.float32),
        "gb1": ws["gb1"].reshape(E, 1).astype(np.float32),
        "gb2": ws["gb2"].reshape(E, 1).astype(np.float32),
        "gb3": ws["gb3"].reshape(C, 1).astype(np.float32),
    }

    in_maps = []
    for k in range(N_CORES):
        lo, hi = ptr[k * SEGS_PER_CORE], ptr[(k + 1) * SEGS_PER_CORE]
        xt4, srow = _prep_core(x[lo:hi], index[lo:hi], counts, k,
                               w_rows, n_quads, R)
        in_maps.append({"xt4": xt4, "srow": srow, **common})

    res = bass_utils.run_bass_kernel_spmd(nc, in_maps, core_ids=list(range(N_CORES)))
    global LAST_RESULTS, LAST_NC, LAST_IN_MAPS
    LAST_RESULTS, LAST_NC, LAST_IN_MAPS = res, nc, in_maps
    out = np.concatenate([res.results[k]["out"] for k in range(N_CORES)], axis=0)
    return out.astype(np.float32)


LAST_RESULTS = None
LAST_NC = None
LAST_IN_MAPS = None


if __name__ == "__main__":
    rng = np.random.default_rng(0)
    idx = np.sort(rng.integers(0, NUM_ELECTIONS, size=N_VOTERS)).astype(np.int64)
    demo = {
        "x": rng.standard_normal((N_VOTERS, C), dtype=np.float32),
        "index": idx,
    }
    for n, sh in (("lW1", (C, E)), ("lW2", (E, E)), ("lW3", (E, E)),
                  ("gW1", (E, E)), ("gW2", (E, E)), ("gW3", (E, C))):
        demo[n] = (rng.standard_normal(sh, dtype=np.float32) * 0.05)
    for n, sh in (("lb1", E), ("lb2", E), ("lb3", E),
                  ("gb1", E), ("gb2", E), ("gb3", C)):
        demo[n] = np.zeros(sh, np.float32)
    out = kernel(**demo)
    print(out.shape, out.dtype, np.isfinite(out).all())


# revision 17
# speedup vs baseline: 433.4232x; 1.0302x over previous
"""DeepSet election model on 8 Trainium2 NeuronCores.

Strategy (differs from the all-reduce hint, exploiting the *sorted* index):
rows are sharded by SEGMENT OWNERSHIP - core k gets every row whose election
id falls in [512k, 512(k+1)).  Every segment then lives entirely on one core,
so no collective is needed at all.

Per core pipeline (all activations bf16, accumulation f32 in PSUM):
  1. L1:   h1T[128emb, rows] = lW1.T @ xT       (xT pre-transposed on host,
           4-way row-tiled K=32 matmuls)
  2. relu1 evac PSUM->SBUF (+lb1 bias, per-partition)        [DVE/ACT split]
  3. L2:   h2pre[rows, emb] = h1T_chunk.T @ lW2 (h1T chunk as stationary)
  4. relu2 evac PSUM->SBUF                                    [DVE/ACT split]
  5. segment sums: per 128-row chunk, one-hot S_row[128rows, 64segs] matmul
     (shipped from host) accumulating into per-window [64seg, 128] PSUM;
     rows are padded per (core,window) to a fixed row count so the chunk ->
     window map is static and identical on every core
  6. deferred local layer 3 (linear, pushed past the segment sum),
     global MLP, log_softmax - all on the tiny [512, 128] per-core tensor.
"""

import math
from contextlib import ExitStack

import numpy as np
import ml_dtypes

import concourse.bass as bass
import concourse.bacc as bacc
import concourse.mybir as mybir
import concourse.tile as tile
from concourse import bass_utils

BF16 = mybir.dt.bfloat16
F32 = mybir.dt.float32
AF = mybir.ActivationFunctionType
ALU = mybir.AluOpType

N_VOTERS = 1048576
NUM_ELECTIONS = 4096
C = 32     # candidates
E = 128    # embedding width
N_CORES = 8
SEGS_PER_CORE = NUM_ELECTIONS // N_CORES   # 512
W_SEGS = 64                                # segments per PSUM window
N_WINDOWS = SEGS_PER_CORE // W_SEGS        # 8

_nb16 = lambda a: np.ascontiguousarray(a).astype(ml_dtypes.bfloat16)


def _build_program(w_rows: int):
    """Build + compile the SPMD Bass program. w_rows = padded rows per
    (core, window); multiple of 512. Identical structure on every core."""
    assert w_rows % 512 == 0
    R = N_WINDOWS * w_rows                  # rows per core
    G = R // 512                            # groups
    n_chunks = R // 128
    cpw = w_rows // 128                     # chunks per window
    assert G % 4 == 0
    n_quads = G // 4

    nc = bacc.Bacc(
        "TRN2",
        target_bir_lowering=False,
        debug=False,
        enable_asserts=True,
        num_devices=N_CORES,
    )

    dt_in = lambda n, sh, dt: nc.dram_tensor(n, sh, dt, kind="ExternalInput").ap()
    xt4 = dt_in("xt4", [n_quads, 128, 512], BF16)
    srowd = dt_in("srow", [n_quads, 128, 16 * W_SEGS], BF16)
    lw1x4 = dt_in("lw1x4", [128, E], BF16)
    lw2 = dt_in("lw2", [E, E], BF16)
    lw3 = dt_in("lw3", [E, E], BF16)
    gw1 = dt_in("gw1", [E, E], BF16)
    gw2 = dt_in("gw2", [E, E], BF16)
    gw3 = dt_in("gw3", [E, C], BF16)
    identf = dt_in("identf", [128, 128], F32)
    lb1d = dt_in("lb1", [E, 1], F32)
    gb1d = dt_in("gb1", [E, 1], F32)
    gb2d = dt_in("gb2", [E, 1], F32)
    gb3d = dt_in("gb3", [C, 1], F32)
    out_ap = nc.dram_tensor("out", [SEGS_PER_CORE, C], F32, kind="ExternalOutput").ap()

    with tile.TileContext(nc) as tc:
        with ExitStack() as octx:
            cpool = octx.enter_context(tc.tile_pool(name="const", bufs=1))
            aggps = octx.enter_context(tc.tile_pool(name="aggps", bufs=1, space="PSUM"))
            tailp = octx.enter_context(tc.tile_pool(name="tail", bufs=2))

            def cload(ap, shape, dtype, tag):
                t = cpool.tile(shape, dtype, tag=tag)
                nc.sync.dma_start(t[:], ap[:])
                return t

            # critical-path constants only; tail-only constants are DMA'd
            # after the main loop so the first x/S_row tiles aren't queued
            # behind them
            w1 = cload(lw1x4, [128, E], BF16, "w1")
            w2 = cload(lw2, [E, E], BF16, "w2")
            lb1 = cload(lb1d, [E, 1], F32, "lb1")

            # aggT: [128 emb, 512 seg] bf16 - segment sums, already transposed
            # for the tail (agg matmul keeps h2 stationary and streams S_row,
            # so PSUM gets aggT windows [128 emb, 64 seg] directly)
            aggT = cpool.tile([128, 4 * E], BF16, tag="aggT")

            lane_flip = 0
            relu2_flip = 0

            def evac_relu(dst, src, bias=None, use_act=None):
                nonlocal lane_flip
                if use_act is None:
                    lane_flip += 1
                    use_act = bool(lane_flip % 2)
                if use_act:
                    nc.scalar.activation(dst, src, AF.Relu,
                                         bias=bias[:] if bias is not None else 0.0)
                else:
                    if bias is not None:
                        nc.vector.tensor_scalar(dst, src, bias[:], 0.0,
                                                ALU.add, ALU.max)
                    else:
                        nc.vector.tensor_scalar_max(dst, src, 0.0)

            def evac_copy(dst, src):
                nonlocal lane_flip
                lane_flip += 1
                if lane_flip % 2:
                    nc.scalar.copy(dst, src)
                else:
                    nc.vector.tensor_copy(dst, src)

            # ================= main per-row loop =================
            with ExitStack() as ictx:
                xtp = ictx.enter_context(tc.tile_pool(name="xt", bufs=5))
                srp = ictx.enter_context(tc.tile_pool(name="sr", bufs=5))
                l1ps = ictx.enter_context(tc.tile_pool(name="l1ps", bufs=2, space="PSUM"))
                h1p = ictx.enter_context(tc.tile_pool(name="h1", bufs=10))
                l2ps = ictx.enter_context(tc.tile_pool(name="l2ps", bufs=3, space="PSUM"))
                h2p = ictx.enter_context(tc.tile_pool(name="h2", bufs=8))

                agg_tile = None

                for q in range(n_quads):
                    xt = xtp.tile([128, 512], BF16, tag="xt")
                    nc.sync.dma_start(xt[:], xt4[q])
                    sr = srp.tile([128, 16 * W_SEGS], BF16, tag="sr")
                    nc.sync.dma_start(sr[:], srowd[q])

                    # L1: two [128,1024] PSUM tiles per quad, 2 groups each
                    h1s = []
                    for h in range(2):
                        l1 = l1ps.tile([128, 1024], F32, tag="l1")
                        for i2 in range(2):
                            i = 2 * h + i2
                            nc.tensor.matmul(
                                l1[:, 512 * i2:512 * i2 + 512],
                                w1[32 * i:32 * i + 32, :],
                                xt[32 * i:32 * i + 32, :],
                                start=True, stop=True, tile_position=(32 * i, 0),
                            )
                        h1 = h1p.tile([128, 1024], BF16, tag="h1")
                        evac_relu(h1[:], l1[:], bias=lb1)
                        h1s.append(h1)

                    # L2 + segment-sum, chunk by chunk (16 chunks per quad)
                    for i in range(4):
                        h1 = h1s[i // 2]
                        hoff = 512 * (i % 2)
                        l2 = l2ps.tile([128, 512], F32, tag="l2")
                        for c4 in range(4):
                            nc.tensor.matmul(
                                l2[:, 128 * c4:128 * c4 + 128],
                                h1[:, hoff + 128 * c4:hoff + 128 * c4 + 128],
                                w2[:],
                                start=True, stop=True,
                            )
                        h2 = h2p.tile([128, 512], BF16, tag="h2")
                        # ACT is ~1.15x faster per op than DVE at 1x; a 4:3
                        # ACT:DVE split on relu2 (with relu1 at 1:1) equalizes
                        # total lane busy (~152us each vs 165/141 at 1:1)
                        relu2_flip += 1
                        evac_relu(h2[:], l2[:],
                                  use_act=(relu2_flip % 7) in (1, 3, 5, 6))
                        for c4 in range(4):
                            cid = (q * 4 + i) * 4 + c4
                            w = cid // cpw
                            first = (cid % cpw == 0)
                            last = (cid % cpw == cpw - 1)
                            if first:
                                agg_tile = aggps.tile([E, W_SEGS], F32, tag="agg")
                            # aggT_w = h2_chunk.T @ S_row: h2 stationary
                            # (FWL-eligible), S_row streamed at N=64 (half the
                            # stream of the S_row-stationary form), and the
                            # output lands pre-transposed for the tail
                            nc.tensor.matmul(
                                agg_tile[:],
                                h2[:, 128 * c4:128 * c4 + 128],
                                sr[:, W_SEGS * (4 * i + c4):
                                   W_SEGS * (4 * i + c4 + 1)],
                                start=first, stop=last,
                            )
                            if last:
                                nc.vector.tensor_copy(
                                    aggT[:, W_SEGS * w:W_SEGS * (w + 1)],
                                    agg_tile[:])

            # tail-only constants (deferred DMAs)
            w3 = cload(lw3, [E, E], BF16, "w3")
            g1w = cload(gw1, [E, E], BF16, "g1w")
            g2w = cload(gw2, [E, E], BF16, "g2w")
            g3w = cload(gw3, [E, C], BF16, "g3w")
            idf = cload(identf, [128, 128], F32, "idf")
            gb1 = cload(gb1d, [E, 1], F32, "gb1")
            gb2 = cload(gb2d, [E, 1], F32, "gb2")
            gb3 = cload(gb3d, [C, 1], F32, "gb3")

            # ---------- tail: deferred layer-3 + global MLP ----------
            with ExitStack() as tctx:
                tailps = tctx.enter_context(
                    tc.tile_pool(name="tailps", bufs=2, space="PSUM"))

                def layerT(rhs_tile, w_tile, func, bias, out_dt, out_cols=E,
                           tag=""):
                    ps = tailps.tile([out_cols, 512], F32, tag="lps")
                    nc.tensor.matmul(ps[:], w_tile[:], rhs_tile[:],
                                     start=True, stop=True)
                    o = tailp.tile([out_cols, 512], out_dt, tag=tag)
                    if func is None:
                        evac_copy(o[:], ps[:])
                    else:
                        nc.scalar.activation(
                            o[:], ps[:], func,
                            bias=bias[:] if bias is not None else 0.0)
                    return o

                a3T = layerT(aggT, w3, None, None, BF16, tag="a3T")
                g1T = layerT(a3T, g1w, AF.Relu, gb1, BF16, tag="g1T")
                g2T = layerT(g1T, g2w, AF.Relu, gb2, BF16, tag="g2T")
                scT = layerT(g2T, g3w, AF.Identity, gb3, F32, out_cols=C,
                             tag="scT")

                # log-softmax, phase-batched so ACT loads the exp table once
                # and the ln table once (alternating Exp/Ln per chunk costs a
                # ~2.7us ACT table load EACH time)
                outsb = tailp.tile([128, 4 * C], F32, tag="outsb")
                xs = tailp.tile([128, 4 * C], F32, tag="xs")
                exs = tailp.tile([128, 4 * C], F32, tag="exs")
                negmax = tailp.tile([128, 4], F32, tag="negmax")
                ssum = tailp.tile([128, 4], F32, tag="ssum")
                lse = tailp.tile([128, 4], F32, tag="lse")
                shift = tailp.tile([128, 4], F32, tag="shift")
                mx = tailp.tile([128, 4], F32, tag="mx")
                for t in range(4):
                    sp = tailps.tile([128, C], F32, tag="sp")
                    nc.tensor.transpose(sp[:], scT[:, 128 * t:128 * t + 128],
                                        idf[:C, :C])
                    nc.vector.tensor_copy(xs[:, C * t:C * (t + 1)], sp[:])
                    nc.vector.tensor_reduce(mx[:, t:t + 1],
                                            xs[:, C * t:C * (t + 1)],
                                            mybir.AxisListType.X, ALU.max)
                for t in range(4):
                    nc.vector.tensor_scalar_mul(negmax[:, t:t + 1],
                                                mx[:, t:t + 1], -1.0)
                    nc.scalar.activation(exs[:, C * t:C * (t + 1)],
                                         xs[:, C * t:C * (t + 1)], AF.Exp,
                                         bias=negmax[:, t:t + 1])
                    nc.vector.reduce_sum(ssum[:, t:t + 1],
                                         exs[:, C * t:C * (t + 1)],
                                         axis=mybir.AxisListType.X)
                nc.scalar.activation(lse[:], ssum[:], AF.Ln)
                nc.vector.tensor_tensor(shift[:], negmax[:], lse[:],
                                        op=ALU.subtract)
                for t in range(4):
                    nc.vector.tensor_scalar_add(outsb[:, C * t:C * (t + 1)],
                                                xs[:, C * t:C * (t + 1)],
                                                shift[:, t:t + 1])

                outv = out_ap.rearrange("(t p) c -> t p c", p=128)
                for t in range(4):
                    nc.sync.dma_start(outv[t], outsb[:, C * t:C * (t + 1)])

    nc.compile()
    return nc, G, R


def _prep_core(x, index_local, counts, core, w_rows, n_quads, R):
    """Per-core xt4 + srow tensors."""
    segs0 = core * SEGS_PER_CORE
    cnt = counts[segs0:segs0 + SEGS_PER_CORE]
    seg_of_row = index_local - segs0

    # destination row: window-contiguous with per-window padding to w_rows
    win_of_row = seg_of_row // W_SEGS
    win_cnt = np.bincount(win_of_row, minlength=N_WINDOWS)
    win_orig_start = np.concatenate(([0], np.cumsum(win_cnt)[:-1]))
    dest = win_of_row * w_rows + (np.arange(len(index_local))
                                  - win_orig_start[win_of_row])
    xpad = np.zeros((R, C), dtype=np.float32)
    xpad[dest] = x
    xt4 = xpad.reshape(n_quads, 4, 512, C).transpose(0, 1, 3, 2).reshape(
        n_quads, 128, 512)

    # per-row one-hot vs window-relative segment id
    d = np.full(R, -(10 ** 6), dtype=np.int64)
    d[dest] = seg_of_row - win_of_row * W_SEGS
    srow = (d[:, None] == np.arange(W_SEGS)[None, :])      # [R, 64]
    n_chunks = R // 128
    srow = srow.reshape(n_chunks, 128, W_SEGS).transpose(1, 0, 2)
    # group 16 chunks (one quad) per DMA tile
    srow = srow.reshape(128, n_quads, 16 * W_SEGS).transpose(1, 0, 2)
    return _nb16(xt4), _nb16(np.ascontiguousarray(srow).astype(np.float32))


def kernel(**inputs) -> np.ndarray:
    x = np.asarray(inputs["x"], dtype=np.float32)
    index = np.asarray(inputs["index"]).astype(np.int64)
    ws = {k: np.asarray(inputs[k], dtype=np.float32)
          for k in ("lW1", "lb1", "lW2", "lb2", "lW3", "lb3",
                    "gW1", "gb1", "gW2", "gb2", "gW3", "gb3")}

    # lb2 enters per-row on the free axis, lb3 would need per-segment counts;
    # both are zero for this model.
    assert not ws["lb2"].any() and not ws["lb3"].any(), \
        "nonzero lb2/lb3 not supported by this kernel"

    if not np.all(index[:-1] <= index[1:]):
        order = np.argsort(index, kind="stable")
        index = index[order]
        x = x[order]

    counts = np.bincount(index, minlength=NUM_ELECTIONS)
    ptr = np.concatenate(([0], np.cumsum(counts)))

    # rows per (core, window), padded to the global max (512-aligned)
    win_rows = counts.reshape(N_CORES * N_WINDOWS, W_SEGS).sum(axis=1)
    w_rows = int(-(-win_rows.max() // 512) * 512)

    nc, G, R = _build_program(w_rows)
    n_quads = G // 4

    common = {
        "lw1x4": _nb16(np.tile(ws["lW1"], (4, 1))),
        "lw2": _nb16(ws["lW2"]),
        "lw3": _nb16(ws["lW3"]),
        "gw1": _nb16(ws["gW1"]),
        "gw2": _nb16(ws["gW2"]),
        "gw3": _nb16(ws["gW3"]),
        "identf": np.eye(128, dtype=np.float32),
        "lb1": ws["lb1"].reshape(E, 1).astype(np.float32),
        "gb1": ws["gb1"].reshape(E, 1).astype(np.float32),
        "gb2": ws["gb2"].reshape(E, 1).astype(np.float32),
        "gb3": ws["gb3"].reshape(C, 1).astype(np.float32),
    }

    in_maps = []
    for k in range(N_CORES):
        lo, hi = ptr[k * SEGS_PER_CORE], ptr[(k + 1) * SEGS_PER_CORE]
        xt4, srow = _prep_core(x[lo:hi], index[lo:hi], counts, k,
                               w_rows, n_quads, R)
        in_maps.append({"xt4": xt4, "srow": srow, **common})

    res = bass_utils.run_bass_kernel_spmd(nc, in_maps, core_ids=list(range(N_CORES)))
    global LAST_RESULTS, LAST_NC, LAST_IN_MAPS
    LAST_RESULTS, LAST_NC, LAST_IN_MAPS = res, nc, in_maps
    out = np.concatenate([res.results[k]["out"] for k in range(N_CORES)], axis=0)
    return out.astype(np.float32)


LAST_RESULTS = None
LAST_NC = None
LAST_IN_MAPS = None


if __name__ == "__main__":
    rng = np.random.default_rng(0)
    idx = np.sort(rng.integers(0, NUM_ELECTIONS, size=N_VOTERS)).astype(np.int64)
    demo = {
        "x": rng.standard_normal((N_VOTERS, C), dtype=np.float32),
        "index": idx,
    }
    for n, sh in (("lW1", (C, E)), ("lW2", (E, E)), ("lW3", (E, E)),
                  ("gW1", (E, E)), ("gW2", (E, E)), ("gW3", (E, C))):
        demo[n] = (rng.standard_normal(sh, dtype=np.float32) * 0.05)
    for n, sh in (("lb1", E), ("lb2", E), ("lb3", E),
                  ("gb1", E), ("gb2", E), ("gb3", C)):
        demo[n] = np.zeros(sh, np.float32)
    out = kernel(**demo)
    print(out.shape, out.dtype, np.isfinite(out).all())
